# revision 1
# baseline (speedup 1.0000x reference)
"""Sparse (top-k pruned) multi-head attention on 8 Trainium2 NeuronCores.

Sharding: batch(2) x head-groups(4 heads) -> 8 cores. Each core:
  - projects q/k/v for its 4 heads (fp16x2 split matmuls for q/k precision,
    f32r for v),
  - per head: scores S^T = K Q^T (fp16 two-pass: khi x [qhi;qlo] stacked-K,
    klo x qhi), exp -> E (f32r), AV with ones-augmented V -> O_full and
    row-sums l, colsum_k = sum_q E[k,q]/l_q via TT-mult + reduce,
  - top-k threshold via branchless multi-threshold bisection on-device,
  - complement correction: C = sum_{dropped k} E v, O = (O_full - C) / l,
  - out-projection partial (tensor-parallel on d_model).
Host sums the per-core partials and adds bo.
"""
import sys
for p in ('/opt/trn_rl_repo', '/opt/pypackages'):
    if p not in sys.path:
        sys.path.insert(0, p)
import numpy as np
from contextlib import ExitStack

import concourse.bass as bass
import concourse.bacc as bacc_mod
import concourse.tile as tile
import concourse.mybir as mybir

dt = mybir.dt
F = mybir.ActivationFunctionType
A = mybir.AluOpType

B, S, DM, H, DK = 2, 2048, 1024, 16, 64
HPC = 4              # heads per core
CPC = HPC * DK       # proj columns per core (256)
KEEP = int(S * 0.9)  # 1843
NDROP_THR = 1842.5   # count(c > t) target boundary
N_CORES = 8
KT = DM // 128       # 8 contraction tiles for projections
NQ = S // 512        # 4 q chunks
NKB = S // 128       # 16 key tiles

_CACHE = {}


def _emit(nc):
    ei = lambda n, s, d: nc.dram_tensor(n, s, d, kind="ExternalInput")
    xqh = ei("xqh", [DM, S], dt.float16); xql = ei("xql", [DM, S], dt.float16)
    xkh = ei("xkh", [DM, S], dt.float16); xkl = ei("xkl", [DM, S], dt.float16)
    xv = ei("xv", [DM, S], dt.float32)
    wqh = ei("wqh", [DM, CPC], dt.float16); wql = ei("wql", [DM, CPC], dt.float16)
    wkh = ei("wkh", [DM, CPC], dt.float16); wkl = ei("wkl", [DM, CPC], dt.float16)
    wv = ei("wv", [DM, CPC], dt.float32)
    wo = ei("wo", [CPC, DM], dt.bfloat16)
    bqi = ei("bqi", [128, 2], dt.float32)
    bki = ei("bki", [128, 2], dt.float32)
    bvi = ei("bvi", [1, CPC], dt.float32)
    out_part = nc.dram_tensor("out_part", [DM, S], dt.float32, kind="ExternalOutput")

    # DRAM scratch for per-head projected q/k halves (fp16)
    s_qhi = nc.dram_tensor("s_qhi", [HPC, 64, S], dt.float16)
    s_qlo = nc.dram_tensor("s_qlo", [HPC, 64, S], dt.float16)
    s_khi = nc.dram_tensor("s_khi", [HPC, 64, S], dt.float16)
    s_klo = nc.dram_tensor("s_klo", [HPC, 64, S], dt.float16)

    with tile.TileContext(nc) as tc, ExitStack() as ctx:
        # long-lived pools
        const_pool = ctx.enter_context(tc.tile_pool(name="const", bufs=1))
        vaug_pool = ctx.enter_context(tc.tile_pool(name="vaug", bufs=64))
        ocat_pool = ctx.enter_context(tc.tile_pool(name="ocat", bufs=2))

        ones_1x128 = const_pool.tile([1, 128], dt.float32, tag="c1")
        nc.gpsimd.memset(ones_1x128[:], 1.0)
        ones_col = const_pool.tile([128, 1], dt.float32, tag="c2")
        nc.gpsimd.memset(ones_col[:], 1.0)
        ones_f32r = const_pool.tile([128, 1], dt.float32r, tag="c3")
        nc.vector.tensor_copy(ones_f32r[:], ones_col[:])
        NTHR = 7
        Jt = const_pool.tile([128, NTHR], dt.float32, tag="c4")
        for j in range(NTHR):
            nc.gpsimd.memset(Jt[:, j:j + 1], float(j + 1))
        bq_sb = const_pool.tile([128, 2], dt.float32, tag="c5")
        nc.sync.dma_start(bq_sb[:], bqi[:, :])
        bk_sb = const_pool.tile([128, 2], dt.float32, tag="c6")
        nc.sync.dma_start(bk_sb[:], bki[:, :])

        v_aug = [[vaug_pool.tile([128, 65], dt.float32r, tag="vaug", name=f"vaug{h}_{kb}")
                  for kb in range(NKB)] for h in range(HPC)]
        ocat = [ocat_pool.tile([128, S], dt.bfloat16, tag="ocat", name=f"ocat{i}") for i in range(2)]

        # ---------------- Phase P: projections ----------------
        with tc.tile_pool(name="pP", bufs=1) as wpool, \
             tc.tile_pool(name="pPx", bufs=9) as xpool, \
             tc.tile_pool(name="pPs", bufs=3) as spool, \
             tc.tile_pool(name="pPps", bufs=2, space="PSUM") as ppsum, \
             tc.tile_pool(name="pPpv", bufs=4, space="PSUM") as ppsum_v:

            # q/k projections (fp16 x2, 3 matmul terms)
            for (xh_d, xl_d, wh_d, wl_d, b_sb, sd_hi, sd_lo) in (
                    (xqh, xql, wqh, wql, bq_sb, s_qhi, s_qlo),
                    (xkh, xkl, wkh, wkl, bk_sb, s_khi, s_klo)):
                wh_t = wpool.tile([128, KT * CPC], dt.float16)
                wl_t = wpool.tile([128, KT * CPC], dt.float16)
                for kt in range(KT):
                    nc.sync.dma_start(wh_t[:, kt * CPC:(kt + 1) * CPC],
                                      wh_d[kt * 128:(kt + 1) * 128, :])
                    nc.sync.dma_start(wl_t[:, kt * CPC:(kt + 1) * CPC],
                                      wl_d[kt * 128:(kt + 1) * 128, :])
                for nt in range(NQ):
                    xh_t = []
                    xl_t = []
                    for kt in range(KT):
                        th = xpool.tile([128, 512], dt.float16, tag="xh")
                        nc.sync.dma_start(th[:], xh_d[kt * 128:(kt + 1) * 128,
                                                      nt * 512:(nt + 1) * 512])
                        xh_t.append(th)
                        tl = xpool.tile([128, 512], dt.float16, tag="xl")
                        nc.sync.dma_start(tl[:], xl_d[kt * 128:(kt + 1) * 128,
                                                      nt * 512:(nt + 1) * 512])
                        xl_t.append(tl)
                    for ct in range(2):
                        pst = ppsum.tile([128, 512], dt.float32, tag="pj")
                        n_mm = KT * 3
                        i_mm = 0
                        for kt in range(KT):
                            wslice = slice(kt * CPC + ct * 128, kt * CPC + ct * 128 + 128)
                            for (wt, xt) in ((wh_t, xh_t[kt]), (wh_t, xl_t[kt]),
                                             (wl_t, xh_t[kt])):
                                nc.tensor.matmul(pst[:], wt[:, wslice], xt[:],
                                                 start=(i_mm == 0),
                                                 stop=(i_mm == n_mm - 1))
                                i_mm += 1
                        full = spool.tile([128, 512], dt.float32, tag="pf")
                        nc.scalar.activation(full[:], pst[:], F.Identity,
                                             bias=b_sb[:, ct:ct + 1], scale=1.0)
                        hi = spool.tile([128, 512], dt.float16, tag="ph")
                        nc.vector.tensor_copy(hi[:], full[:])
                        tmp = spool.tile([128, 512], dt.float32, tag="pt")
                        nc.vector.tensor_tensor(out=tmp[:], in0=full[:], in1=hi[:],
                                                op=A.subtract)
                        lo = spool.tile([128, 512], dt.float16, tag="pl")
                        nc.vector.tensor_copy(lo[:], tmp[:])
                        for hh in range(2):
                            head = ct * 2 + hh
                            rows = slice(hh * 64, hh * 64 + 64)
                            cols = slice(nt * 512, nt * 512 + 512)
                            nc.sync.dma_start(sd_hi[head, :, cols], hi[rows, :])
                            nc.sync.dma_start(sd_lo[head, :, cols], lo[rows, :])

            # v projection (f32r)
            wv_raw = wpool.tile([128, KT * CPC], dt.float32)
            for kt in range(KT):
                nc.sync.dma_start(wv_raw[:, kt * CPC:(kt + 1) * CPC],
                                  wv[kt * 128:(kt + 1) * 128, :])
            wv_r = wpool.tile([128, KT * CPC], dt.float32r)
            nc.vector.tensor_copy(wv_r[:], wv_raw[:])
            bv_sb = wpool.tile([1, CPC], dt.float32)
            nc.sync.dma_start(bv_sb[:], bvi[:, :])
            ps_bv = ppsum.tile([128, CPC], dt.float32, tag="bv")
            nc.tensor.matmul(ps_bv[:], ones_1x128[:], bv_sb[:], start=True, stop=True)
            bv_bc = wpool.tile([128, CPC], dt.float32)
            nc.vector.tensor_copy(bv_bc[:], ps_bv[:])

            for sg in range(4):          # groups of 4 key-tiles
                ps_v = [ppsum_v.tile([128, CPC], dt.float32, tag="pv", name=f"psv{st}") for st in range(4)]
                for kt in range(KT):
                    xv_t = xpool.tile([128, 512], dt.float32, tag="xv")
                    nc.sync.dma_start(xv_t[:], xv[kt * 128:(kt + 1) * 128,
                                                  sg * 512:(sg + 1) * 512])
                    xv_r = xpool.tile([128, 512], dt.float32r, tag="xvr")
                    nc.vector.tensor_copy(xv_r[:], xv_t[:])
                    for st in range(4):
                        nc.tensor.matmul(ps_v[st][:], xv_r[:, st * 128:(st + 1) * 128],
                                         wv_r[:, kt * CPC:(kt + 1) * CPC],
                                         start=(kt == 0), stop=(kt == KT - 1))
                for st in range(4):
                    kb = sg * 4 + st
                    for h in range(HPC):
                        nc.vector.tensor_tensor(
                            out=v_aug[h][kb][:, 0:64], in0=ps_v[st][:, h * 64:h * 64 + 64],
                            in1=bv_bc[:, h * 64:h * 64 + 64], op=A.add)
                        nc.vector.tensor_copy(v_aug[h][kb][:, 64:65], ones_f32r[:])

        # ---------------- Phase A: per-head attention ----------------
        with tc.tile_pool(name="qk", bufs=1) as qkpool, \
             tc.tile_pool(name="E", bufs=16) as epool, \
             tc.tile_pool(name="att", bufs=2) as apool, \
             tc.tile_pool(name="att1", bufs=1) as apool1, \
             tc.tile_pool(name="ofp", bufs=4) as ofpool, \
             tc.tile_pool(name="vdp", bufs=16) as vdpool, \
             tc.tile_pool(name="bis", bufs=2) as bpool, \
             tc.tile_pool(name="aps", bufs=2, space="PSUM") as score_ps, \
             tc.tile_pool(name="avps", bufs=4, space="PSUM") as av_ps:

            for h in range(HPC):
                qstack = qkpool.tile([128, S], dt.float16, tag="qstack")
                nc.sync.dma_start(qstack[0:64, :], s_qhi[h, :, :])
                nc.sync.dma_start(qstack[64:128, :], s_qlo[h, :, :])
                kdup = qkpool.tile([128, S], dt.float16, tag="kdup")
                nc.sync.dma_start(kdup[0:64, :], s_khi[h, :, :])
                nc.sync.dma_start(kdup[64:128, :], s_khi[h, :, :])
                klo_t = qkpool.tile([64, S], dt.float16, tag="klo")
                nc.sync.dma_start(klo_t[:], s_klo[h, :, :])

                e_t = []
                av = [av_ps.tile([65, 512], dt.float32, tag="av", name=f"av{qb}") for qb in range(NQ)]
                for kb in range(NKB):
                    et = epool.tile([128, S], dt.float32r, tag="E")
                    e_t.append(et)
                    kcols = slice(kb * 128, kb * 128 + 128)
                    for half in range(2):
                        sc = score_ps.tile([128, 1024], dt.float32, tag="sc")
                        for qq in range(2):
                            qs = slice((half * 2 + qq) * 512, (half * 2 + qq) * 512 + 512)
                            ps_slice = sc[:, qq * 512:(qq + 1) * 512]
                            nc.tensor.matmul(ps_slice, kdup[:, kcols], qstack[:, qs],
                                             start=True, stop=False)
                            nc.tensor.matmul(ps_slice, klo_t[0:64, kcols],
                                             qstack[0:64, qs], start=False, stop=True)
                        nc.scalar.activation(et[:, half * 1024:(half + 1) * 1024],
                                             sc[:], F.Exp, bias=0.0, scale=0.125)
                    for qb in range(NQ):
                        nc.tensor.matmul(av[qb][:], v_aug[h][kb][:],
                                         et[:, qb * 512:(qb + 1) * 512],
                                         start=(kb == 0), stop=(kb == NKB - 1))

                # l, r, broadcast of r
                o_full = [ofpool.tile([65, 512], dt.float32, tag="ofull", name=f"of{qb}")
                          for qb in range(NQ)]
                rb = apool1.tile([128, S], dt.float32, tag="rb")
                for qb in range(NQ):
                    nc.vector.tensor_copy(o_full[qb][:], av[qb][:])
                for qb in range(NQ):
                    qs = slice(qb * 512, (qb + 1) * 512)
                    nc.vector.reciprocal(rb[0:1, qs], o_full[qb][64:65, :])
                    ps_bc = score_ps.tile([128, 512], dt.float32, tag="sc")
                    nc.tensor.matmul(ps_bc[:], ones_1x128[:], rb[0:1, qs],
                                     start=True, stop=True)
                    nc.vector.tensor_copy(rb[:, qs], ps_bc[:])

                # colsum
                cs = apool1.tile([128, NKB], dt.float32, tag="cs")
                cs_a = apool1.tile([128, NKB], dt.float32, tag="csa")
                prod = apool1.tile([128, 1024], dt.float32, tag="prod")
                for kb in range(NKB):
                    for hf in range(2):
                        sl = slice(hf * 1024, (hf + 1) * 1024)
                        nc.vector.tensor_tensor(
                            out=prod[:], in0=e_t[kb][:, sl].bitcast(dt.float32),
                            in1=rb[:, sl], op=A.mult)
                        dst = cs_a if hf == 0 else cs
                        nc.vector.reduce_sum(dst[:, kb:kb + 1], prod[:],
                                             axis=mybir.AxisListType.X)
                nc.vector.tensor_tensor(out=cs[:], in0=cs[:], in1=cs_a[:], op=A.add)

                # bisection for top-k threshold: c in (0, 2)
                lw = bpool.tile([1, 2], dt.float32, tag="lw")
                nc.gpsimd.memset(lw[:, 0:1], 0.0)
                nc.gpsimd.memset(lw[:, 1:2], 2.0 / (NTHR + 1))
                cmp_scr = apool1.tile([128, NKB], dt.float32, tag="cmpscr")
                NPH = 12
                for ph in range(NPH):
                    ps_lw = score_ps.tile([128, 2], dt.float32, tag="sc")
                    nc.tensor.matmul(ps_lw[:], ones_1x128[:], lw[:], start=True, stop=True)
                    lw_bc = bpool.tile([128, 2], dt.float32, tag="lwbc")
                    nc.vector.tensor_copy(lw_bc[:], ps_lw[:])
                    Tt = bpool.tile([128, NTHR], dt.float32, tag="T")
                    nc.vector.tensor_scalar(out=Tt[:], in0=Jt[:],
                                            scalar1=lw_bc[:, 1:2], scalar2=lw_bc[:, 0:1],
                                            op0=A.mult, op1=A.add)
                    cnts = bpool.tile([128, NTHR], dt.float32, tag="cnts")
                    for j in range(NTHR):
                        nc.vector.tensor_scalar(out=cmp_scr[:], in0=cs[:],
                                                scalar1=Tt[:, j:j + 1], scalar2=None,
                                                op0=A.is_gt, op1=A.add,
                                                accum_out=cnts[:, j:j + 1])
                    ps_cnt = score_ps.tile([1, NTHR], dt.float32, tag="sc")
                    nc.tensor.matmul(ps_cnt[:], ones_col[:], cnts[:], start=True, stop=True)
                    cnt_sb = bpool.tile([1, NTHR], dt.float32, tag="cntsb")
                    nc.vector.tensor_copy(cnt_sb[:], ps_cnt[:])
                    ge = bpool.tile([1, NTHR], dt.float32, tag="ge")
                    nc.vector.tensor_scalar(out=ge[:], in0=cnt_sb[:], scalar1=NDROP_THR,
                                            scalar2=None, op0=A.is_gt)
                    m_t = bpool.tile([1, 1], dt.float32, tag="m")
                    nc.vector.reduce_sum(m_t[:], ge[:], axis=mybir.AxisListType.X)
                    lw2 = bpool.tile([1, 2], dt.float32, tag="lw")
                    nc.vector.tensor_scalar(out=lw2[:, 0:1], in0=m_t[:],
                                            scalar1=lw[:, 1:2], scalar2=lw[:, 0:1],
                                            op0=A.mult, op1=A.add)
                    nc.vector.tensor_scalar(out=lw2[:, 1:2], in0=lw[:, 1:2],
                                            scalar1=1.0 / (NTHR + 1), scalar2=None,
                                            op0=A.mult)
                    lw = lw2

                # final threshold -> drop mask
                ps_t = score_ps.tile([128, 2], dt.float32, tag="sc")
                nc.tensor.matmul(ps_t[:], ones_1x128[:], lw[:], start=True, stop=True)
                t_bc = bpool.tile([128, 2], dt.float32, tag="lwbc")
                nc.vector.tensor_copy(t_bc[:], ps_t[:])
                m_keep = apool1.tile([128, NKB], dt.float32, tag="mkeep")
                nc.vector.tensor_scalar(out=m_keep[:], in0=cs[:], scalar1=t_bc[:, 0:1],
                                        scalar2=None, op0=A.is_gt)
                m_drop = apool1.tile([128, NKB], dt.float32, tag="mdrop")
                nc.vector.tensor_scalar(out=m_drop[:], in0=m_keep[:], scalar1=-1.0,
                                        scalar2=1.0, op0=A.mult, op1=A.add)

                # complement correction
                cps = [av_ps.tile([65, 512], dt.float32, tag="av", name=f"cps{qb}") for qb in range(NQ)]
                vd = []
                for kb in range(NKB):
                    vdt = vdpool.tile([128, 65], dt.float32r, tag="vd")
                    nc.vector.tensor_scalar(out=vdt[:], in0=v_aug[h][kb][:],
                                            scalar1=m_drop[:, kb:kb + 1], scalar2=None,
                                            op0=A.mult)
                    vd.append(vdt)
                for kb in range(NKB):
                    for qb in range(NQ):
                        nc.tensor.matmul(cps[qb][:], vd[kb][:],
                                         e_t[kb][:, qb * 512:(qb + 1) * 512],
                                         start=(kb == 0), stop=(kb == NKB - 1))

                # O = (O_full - C) * r  -> bf16 into ocat
                tile_idx, rows = h // 2, h % 2
                for qb in range(NQ):
                    qs = slice(qb * 512, qb * 512 + 512)
                    t1 = apool.tile([64, 512], dt.float32, tag="t1")
                    nc.vector.tensor_tensor(out=t1[:], in0=o_full[qb][0:64, :],
                                            in1=cps[qb][0:64, :], op=A.subtract)
                    if rows == 0:
                        nc.vector.tensor_tensor(out=ocat[tile_idx][0:64, qs], in0=t1[:],
                                                in1=rb[0:64, qs], op=A.mult)
                    else:
                        t2 = apool.tile([64, 512], dt.bfloat16, tag="t2")
                        nc.vector.tensor_tensor(out=t2[:], in0=t1[:],
                                                in1=rb[0:64, qs], op=A.mult)
                        nc.sync.dma_start(ocat[tile_idx][64:128, qs], t2[:])

        # ---------------- Phase O: out-projection partial ----------------
        with tc.tile_pool(name="oW", bufs=1) as wopool, \
             tc.tile_pool(name="oS", bufs=3) as ospool, \
             tc.tile_pool(name="ops", bufs=3, space="PSUM") as opsum:
            wo_t = wopool.tile([128, 2 * DM], dt.bfloat16)
            for ct in range(2):
                nc.sync.dma_start(wo_t[:, ct * DM:(ct + 1) * DM],
                                  wo[ct * 128:(ct + 1) * 128, :])
            for ot in range(DM // 128):
                for qb in range(NQ):
                    pso = opsum.tile([128, 512], dt.float32, tag="o")
                    for ct in range(2):
                        nc.tensor.matmul(pso[:],
                                         wo_t[:, ct * DM + ot * 128: ct * DM + ot * 128 + 128],
                                         ocat[ct][:, qb * 512:(qb + 1) * 512],
                                         start=(ct == 0), stop=(ct == 1))
                    osb = ospool.tile([128, 512], dt.float32, tag="osb")
                    nc.scalar.copy(osb[:], pso[:])
                    nc.sync.dma_start(out_part[ot * 128:(ot + 1) * 128,
                                               qb * 512:(qb + 1) * 512], osb[:])
    nc.compile()
    return nc


def _get_nc():
    if "nc" not in _CACHE:
        nc = bacc_mod.Bacc('TRN2', target_bir_lowering=False)
        _emit(nc)
        _CACHE["nc"] = nc
    return _CACHE["nc"]


def _split16(x):
    hi = x.astype(np.float16)
    lo = (x - hi.astype(np.float32)).astype(np.float16)
    return hi, lo


def kernel(q, k, v, Wq, bq, Wk, bk, Wv, bv, Wo, bo):
    q, k, v = (np.asarray(a, np.float32) for a in (q, k, v))
    Wq, bq, Wk, bk, Wv, bv, Wo, bo = (np.asarray(a, np.float32) for a in
                                      (Wq, bq, Wk, bk, Wv, bv, Wo, bo))
    nc = _get_nc()

    xt = {}
    for b in range(B):
        xq = np.ascontiguousarray(q[b].T)
        xk = np.ascontiguousarray(k[b].T)
        xt[b] = (_split16(xq), _split16(xk), np.ascontiguousarray(v[b].T))

    in_maps = []
    for core in range(N_CORES):
        b = core // 4
        h0 = (core % 4) * HPC
        cols = slice(h0 * DK, (h0 + HPC) * DK)
        (qh, ql), (kh, kl), xv = xt[b]
        wq_s = np.ascontiguousarray(Wq[cols].T)
        wk_s = np.ascontiguousarray(Wk[cols].T)
        wqh, wql_ = _split16(wq_s)
        wkh, wkl_ = _split16(wk_s)
        in_maps.append({
            "xqh": qh, "xql": ql, "xkh": kh, "xkl": kl, "xv": xv,
            "wqh": wqh, "wql": wql_, "wkh": wkh, "wkl": wkl_,
            "wv": np.ascontiguousarray(Wv[cols].T),
            "wo": np.ascontiguousarray(Wo[:, cols].T).astype(
                np.dtype(dt.np(dt.bfloat16))),
            "bqi": np.ascontiguousarray(bq[cols].reshape(2, 128).T),
            "bki": np.ascontiguousarray(bk[cols].reshape(2, 128).T),
            "bvi": np.ascontiguousarray(bv[cols].reshape(1, CPC)),
        })

    from concourse.bass_utils import run_bass_kernel_spmd
    _CACHE["last_in_maps"] = in_maps
    res = run_bass_kernel_spmd(nc, in_maps, core_ids=list(range(N_CORES)))
    _CACHE["last_res"] = res

    out = np.zeros((B, S, DM), np.float32)
    for core in range(N_CORES):
        b = core // 4
        out[b] += res.results[core]["out_part"].T
    out += bo.reshape(1, 1, DM)
    return out



# revision 11
# speedup vs baseline: 1.2340x; 1.2340x over previous
"""Sparse (top-k pruned) multi-head attention on 8 Trainium2 NeuronCores.

Sharding: batch(2) x head-groups(4 heads) -> 8 cores.

Per core (4 heads h=0..3, pairs ct=h//2):
  P: q-proj 1-term fp16; k-proj 3-term fp16 hi/lo (precision carrier);
     v-proj 1-term fp16 -> v_aug [128,16,65] (ones col).
     Scores operands: kstack_h = [khi;klo] dims stacked, qdup_h = [q;q].
  A (per head): scores S^T = kstack^T qdup (one 128-contraction fp16 mm
     per (kb,qs)); ACT exp (scale=1/8) -> E fp16 + f32 accum csA (exact
     unnormalized colsum); AV1 = E @ [v;1] -> O_full + row sums l;
     r = approx_recip(broadcast l); colsum = rbar*csA + sum(E*(r-rbar))
     (compensated, near-f32); C_row = PE-transpose+broadcast of colsums;
     128-thresholds-per-partition bisection x4 phases -> exact top-1843
     threshold; complement AV2 = E @ (v*drop); O = (O_full - C)*r.
  O: out-projection partial (tensor-parallel on d_model), f32 partials.
Host sums per-core partials and adds bo.
"""
import sys
for p in ('/opt/trn_rl_repo', '/opt/pypackages'):
    if p not in sys.path:
        sys.path.insert(0, p)
import numpy as np
from contextlib import ExitStack

import concourse.bass as bass
import concourse.bacc as bacc_mod
import concourse.bass_isa as bass_isa
import concourse.tile as tile
import concourse.mybir as mybir
from concourse import library_config

dt = mybir.dt
F = mybir.ActivationFunctionType
A = mybir.AluOpType
AX = mybir.AxisListType

B, S, DM, H, DK = 2, 2048, 1024, 16, 64
HPC = 4              # heads per core
CPC = HPC * DK       # 256 proj columns per core
KEEP = int(S * 0.9)  # 1843
N_CORES = 8
KT = DM // 128       # 8 contraction tiles for projections
NQ = S // 512        # 4 query chunks
NKB = S // 128       # 16 key tiles
NPH = 4              # bisection phases (129-way each)
NDROP_THR = float(KEEP) - 0.5   # count(c > t) target boundary: 1842.5

_CACHE = {}


def _emit(nc):
    ei = lambda n, s, d: nc.dram_tensor(n, s, d, kind="ExternalInput")
    xq = ei("xq", [DM, S], dt.float16)
    xkh = ei("xkh", [DM, S], dt.float16)
    xkl = ei("xkl", [DM, S], dt.float16)
    xv = ei("xv", [DM, S], dt.float16)
    wq = ei("wq", [DM, CPC], dt.float16)
    wkh = ei("wkh", [DM, CPC], dt.float16)
    wkl = ei("wkl", [DM, CPC], dt.float16)
    wv = ei("wv", [DM, CPC], dt.float16)
    wo = ei("wo", [CPC, DM], dt.float16)
    bqi = ei("bqi", [128, 2], dt.float32)
    bki = ei("bki", [128, 2], dt.float32)
    bvi = ei("bvi", [1, CPC], dt.float32)
    iota_in = ei("iota1", [128, 1], dt.float32)   # values 1..128
    ident_in = ei("ident", [128, 128], dt.float32)
    out_part = nc.dram_tensor("out_part", [DM, S], dt.float32, kind="ExternalOutput")

    with tile.TileContext(nc) as tc, ExitStack() as ctx:
        nc.gpsimd.load_library(library_config.attn)

        # ---------- long-lived pools ----------
        cpool = ctx.enter_context(tc.tile_pool(name="const", bufs=1))
        qkpool = ctx.enter_context(tc.tile_pool(name="qk", bufs=1))
        vpool = ctx.enter_context(tc.tile_pool(name="vaug", bufs=1))
        ocpool = ctx.enter_context(tc.tile_pool(name="ocat", bufs=1))
        # psum pools (total <= 8 banks): scores 2x2 + av1 2x1 + misc 2x1
        misc_ps = ctx.enter_context(tc.tile_pool(name="mps", bufs=2, space="PSUM"))

        ones_1x128 = cpool.tile([1, 128], dt.float32, tag="c1")
        nc.vector.memset(ones_1x128[:], 1.0)
        iota_t = cpool.tile([128, 1], dt.float32, tag="c2")
        nc.sync.dma_start(iota_t[:], iota_in[:, :])
        ident_t = cpool.tile([128, 128], dt.float32, tag="c3")
        nc.sync.dma_start(ident_t[:], ident_in[:, :])
        bq_sb = cpool.tile([128, 2], dt.float32, tag="c4")
        nc.sync.dma_start(bq_sb[:], bqi[:, :])
        bk_sb = cpool.tile([128, 2], dt.float32, tag="c5")
        nc.sync.dma_start(bk_sb[:], bki[:, :])

        qdup = [qkpool.tile([128, S], dt.float16, tag=f"qd{h}", name=f"qd{h}")
                for h in range(HPC)]
        kstack = [qkpool.tile([128, S], dt.float16, tag=f"ks{h}", name=f"ks{h}")
                  for h in range(HPC)]
        v_aug = [vpool.tile([128, NKB, 65], dt.float16, tag=f"va{h}", name=f"va{h}")
                 for h in range(HPC)]
        ocat = [ocpool.tile([128, S], dt.float16, tag=f"oc{i}", name=f"oc{i}")
                for i in range(2)]

        # ---------------- Phase P: projections ----------------
        with tc.tile_pool(name="pW", bufs=1) as wpool, \
             tc.tile_pool(name="pX", bufs=4) as xpool, \
             tc.tile_pool(name="pT", bufs=2) as tpool, \
             tc.tile_pool(name="pPs", bufs=4, space="PSUM") as pj_ps:

            wq_t = wpool.tile([128, KT * CPC], dt.float16)
            wkh_t = wpool.tile([128, KT * CPC], dt.float16)
            wkl_t = wpool.tile([128, KT * CPC], dt.float16)
            wv_t = wpool.tile([128, KT * CPC], dt.float16)
            for kt in range(KT):
                sl = slice(kt * CPC, (kt + 1) * CPC)
                rows = slice(kt * 128, (kt + 1) * 128)
                nc.sync.dma_start(wq_t[:, sl], wq[rows, :])
                nc.sync.dma_start(wkh_t[:, sl], wkh[rows, :])
                nc.sync.dma_start(wkl_t[:, sl], wkl[rows, :])
                nc.sync.dma_start(wv_t[:, sl], wv[rows, :])
            bv_sb = wpool.tile([1, CPC], dt.float32)
            nc.sync.dma_start(bv_sb[:], bvi[:, :])
            ps_bv = misc_ps.tile([128, 512], dt.float32, tag="m")
            nc.tensor.matmul(ps_bv[:, 0:CPC], ones_1x128[:], bv_sb[:],
                             start=True, stop=True)
            bv_bc = wpool.tile([128, CPC], dt.float32)
            nc.vector.tensor_copy(bv_bc[:], ps_bv[:, 0:CPC])

            # ---- q projection (1-term) + k projection (3-term hi/lo) ----
            for nt in range(NQ):
                qs = slice(nt * 512, (nt + 1) * 512)
                xq_t = []
                xkh_t = []
                xkl_t = []
                for kt in range(KT):
                    rows = slice(kt * 128, (kt + 1) * 128)
                    t1 = xpool.tile([128, 512], dt.float16, tag="xq")
                    nc.sync.dma_start(t1[:], xq[rows, qs])
                    xq_t.append(t1)
                    t2 = xpool.tile([128, 512], dt.float16, tag="xkh")
                    nc.sync.dma_start(t2[:], xkh[rows, qs])
                    xkh_t.append(t2)
                    t3 = xpool.tile([128, 512], dt.float16, tag="xkl")
                    nc.sync.dma_start(t3[:], xkl[rows, qs])
                    xkl_t.append(t3)
                for ct in range(2):
                    hA, hB = 2 * ct, 2 * ct + 1
                    # q: single term
                    psq = pj_ps.tile([128, 512], dt.float32, tag="pj")
                    for kt in range(KT):
                        wsl = slice(kt * CPC + ct * 128, kt * CPC + ct * 128 + 128)
                        nc.tensor.matmul(psq[:], wq_t[:, wsl], xq_t[kt][:],
                                         start=(kt == 0), stop=(kt == KT - 1))
                    # qdup: rows 0:64 then DMA-duplicate to rows 64:128
                    nc.vector.tensor_scalar(
                        out=qdup[hA][0:64, qs], in0=psq[0:64, :],
                        scalar1=bq_sb[0:64, ct:ct + 1], scalar2=None, op0=A.add)
                    nc.vector.tensor_scalar(
                        out=qdup[hB][0:64, qs], in0=psq[64:128, :],
                        scalar1=bq_sb[64:128, ct:ct + 1], scalar2=None, op0=A.add)
                    nc.sync.dma_start(qdup[hA][64:128, qs], qdup[hA][0:64, qs])
                    nc.sync.dma_start(qdup[hB][64:128, qs], qdup[hB][0:64, qs])

                    # k: 3 terms -> f32 psum
                    psk = pj_ps.tile([128, 512], dt.float32, tag="pj")
                    i_mm = 0
                    for kt in range(KT):
                        wsl = slice(kt * CPC + ct * 128, kt * CPC + ct * 128 + 128)
                        for (wt, xt) in ((wkh_t, xkh_t[kt]), (wkh_t, xkl_t[kt]),
                                         (wkl_t, xkh_t[kt])):
                            nc.tensor.matmul(psk[:], wt[:, wsl], xt[:],
                                             start=(i_mm == 0),
                                             stop=(i_mm == 3 * KT - 1))
                            i_mm += 1
                    # kA: hi aligned rows 0:64; lo via tmp + shift-DMA
                    nc.vector.tensor_scalar(
                        out=kstack[hA][0:64, qs], in0=psk[0:64, :],
                        scalar1=bk_sb[0:64, ct:ct + 1], scalar2=None, op0=A.add)
                    tfA = tpool.tile([128, 512], dt.float32, tag="tf")
                    nc.vector.tensor_scalar(
                        out=tfA[0:64, :], in0=psk[0:64, :],
                        scalar1=bk_sb[0:64, ct:ct + 1], scalar2=None, op0=A.add)
                    tA16 = tpool.tile([128, 512], dt.float16, tag="t16")
                    nc.vector.tensor_tensor(out=tA16[0:64, :], in0=tfA[0:64, :],
                                            in1=kstack[hA][0:64, qs], op=A.subtract)
                    nc.sync.dma_start(kstack[hA][64:128, qs], tA16[0:64, :])
                    # kB: hi into tmp rows 64:128 (aligned), shift-DMA to rows 0:64;
                    #     lo aligned rows 64:128
                    tB16 = tpool.tile([128, 512], dt.float16, tag="t16")
                    nc.vector.tensor_scalar(
                        out=tB16[64:128, :], in0=psk[64:128, :],
                        scalar1=bk_sb[64:128, ct:ct + 1], scalar2=None, op0=A.add)
                    nc.sync.dma_start(kstack[hB][0:64, qs], tB16[64:128, :])
                    tfB = tpool.tile([128, 512], dt.float32, tag="tf")
                    nc.vector.tensor_scalar(
                        out=tfB[64:128, :], in0=psk[64:128, :],
                        scalar1=bk_sb[64:128, ct:ct + 1], scalar2=None, op0=A.add)
                    nc.vector.tensor_tensor(out=kstack[hB][64:128, qs],
                                            in0=tfB[64:128, :], in1=tB16[64:128, :],
                                            op=A.subtract)

            # ---- v projection (1-term fp16) ----
            for tb in range(NKB):
                tsl = slice(tb * 128, (tb + 1) * 128)
                psv = pj_ps.tile([128, 512], dt.float32, tag="pj")
                for kt in range(KT):
                    rows = slice(kt * 128, (kt + 1) * 128)
                    xv_t = xpool.tile([128, 128], dt.float16, tag="xv")
                    nc.sync.dma_start(xv_t[:], xv[rows, tsl])
                    nc.tensor.matmul(psv[:, 0:CPC], xv_t[:],
                                     wv_t[:, kt * CPC:(kt + 1) * CPC],
                                     start=(kt == 0), stop=(kt == KT - 1))
                for h in range(HPC):
                    nc.vector.tensor_tensor(
                        out=v_aug[h][:, tb, 0:64], in0=psv[:, h * 64:(h + 1) * 64],
                        in1=bv_bc[:, h * 64:(h + 1) * 64], op=A.add)
            for h in range(HPC):
                nc.vector.memset(v_aug[h][:, :, 64:65], 1.0)

        # ---------------- Phase A: per-head attention ----------------
        score_ps = ctx.enter_context(tc.tile_pool(name="scps", bufs=2, space="PSUM"))
        av1_ps = ctx.enter_context(tc.tile_pool(name="avps", bufs=2, space="PSUM"))
        epool = ctx.enter_context(tc.tile_pool(name="E", bufs=16))
        ofpool = ctx.enter_context(tc.tile_pool(name="ofull", bufs=2))
        rbpool = ctx.enter_context(tc.tile_pool(name="rb", bufs=2))
        w16pool = ctx.enter_context(tc.tile_pool(name="w16", bufs=2))
        crpool = ctx.enter_context(tc.tile_pool(name="crow", bufs=1))
        scrpool = ctx.enter_context(tc.tile_pool(name="scr", bufs=2))
        cspool = ctx.enter_context(tc.tile_pool(name="cs", bufs=2))
        lspool = ctx.enter_context(tc.tile_pool(name="ls", bufs=2))
        smpool = ctx.enter_context(tc.tile_pool(name="sm", bufs=4))
        vdpool = ctx.enter_context(tc.tile_pool(name="vd", bufs=2))
        for h in range(HPC):
            e_t = []
            csA2 = cspool.tile([128, 2 * NKB], dt.float32, tag="csA2")
            av1 = [av1_ps.tile([65, 512], dt.float32, tag="av", name=f"av{h}_{qb}")
                   for qb in range(2)]
            for kb in range(NKB):
                et = epool.tile([128, S], dt.float16, tag="E", name=f"E{h}_{kb}")
                e_t.append(et)
                kcols = slice(kb * 128, (kb + 1) * 128)
                for half in range(2):
                    sc = score_ps.tile([128, 1024], dt.float32, tag="sc")
                    for qq in range(2):
                        qs = slice((half * 2 + qq) * 512, (half * 2 + qq) * 512 + 512)
                        nc.tensor.matmul(sc[:, qq * 512:(qq + 1) * 512],
                                         kstack[h][:, kcols], qdup[h][:, qs],
                                         start=True, stop=True)
                    nc.scalar.activation(
                        et[:, half * 1024:(half + 1) * 1024], sc[:], F.Exp,
                        bias=0.0, scale=0.125,
                        accum_out=csA2[:, 2 * kb + half:2 * kb + half + 1])
                # AV1 for qb 0,1 accumulates during the scores pass
                for qb in range(2):
                    nc.tensor.matmul(av1[qb][:], v_aug[h][:, kb, :],
                                     e_t[kb][:, qb * 512:(qb + 1) * 512],
                                     start=(kb == 0), stop=(kb == NKB - 1))

            # drain av1 qb 0,1 -> o_full, then run qb 2,3
            o_full = ofpool.tile([65, S], dt.float32, tag="of", name=f"of{h}")
            for qb in range(2):
                nc.vector.tensor_copy(o_full[:, qb * 512:(qb + 1) * 512], av1[qb][:])
            av2r = [av1_ps.tile([65, 512], dt.float32, tag="av", name=f"av2_{h}_{qb}")
                    for qb in range(2)]
            for kb in range(NKB):
                for qb in range(2):
                    nc.tensor.matmul(av2r[qb][:], v_aug[h][:, kb, :],
                                     e_t[kb][:, (2 + qb) * 512:(3 + qb) * 512],
                                     start=(kb == 0), stop=(kb == NKB - 1))
            for qb in range(2):
                nc.vector.tensor_copy(o_full[:, (2 + qb) * 512:(3 + qb) * 512],
                                      av2r[qb][:])

            # rb = approx 1/l broadcast to 128 partitions; rbar; w16 = r - rbar
            rb = rbpool.tile([128, S], dt.float32, tag="rb", name=f"rb{h}")
            l_sb = lspool.tile([1, S], dt.float32, tag="lsf")
            nc.sync.dma_start(l_sb[:], o_full[64:65, :])
            for qb in range(NQ):
                qs = slice(qb * 512, (qb + 1) * 512)
                ps_l = misc_ps.tile([128, 512], dt.float32, tag="m")
                nc.tensor.matmul(ps_l[:], ones_1x128[:], l_sb[0:1, qs],
                                 start=True, stop=True)
                nc.vector.reciprocal(rb[:, qs], ps_l[:])
            rmax = smpool.tile([128, 1], dt.float32, tag="s1")
            nc.vector.tensor_reduce(rmax[:], rb[:], axis=AX.X, op=A.max)
            rbar = smpool.tile([128, 1], dt.float32, tag="s1")
            nc.vector.tensor_scalar(out=rbar[:], in0=rmax[:], scalar1=0.97,
                                    scalar2=None, op0=A.mult)
            w16 = w16pool.tile([128, S], dt.float16, tag="w16", name=f"w16{h}")
            nc.vector.tensor_scalar(out=w16[:], in0=rb[:], scalar1=rbar[:, 0:1],
                                    scalar2=None, op0=A.subtract)

            # compensated colsum: cs = rbar*csA + sum_q E*(r-rbar)
            csB = cspool.tile([128, NKB], dt.float32, tag="csB")
            for kb in range(NKB):
                scr = scrpool.tile([128, S], dt.float16, tag="sc16")
                nc.vector.tensor_tensor(out=scr[:], in0=e_t[kb][:], in1=w16[:],
                                        op=A.mult)
                nc.vector.reduce_sum(csB[:, kb:kb + 1], scr[:], axis=AX.X)
            cs = cspool.tile([128, 32], dt.float32, tag="cs")
            nc.vector.tensor_tensor(out=cs[:, 0:NKB],
                                    in0=csA2[:, 0:2 * NKB:2],
                                    in1=csA2[:, 1:2 * NKB:2], op=A.add)
            nc.vector.tensor_scalar(out=cs[:, 0:NKB], in0=cs[:, 0:NKB],
                                    scalar1=rbar[:, 0:1], scalar2=None, op0=A.mult)
            nc.vector.tensor_tensor(out=cs[:, 0:NKB], in0=cs[:, 0:NKB],
                                    in1=csB[:], op=A.add)

            # C_row: transpose cs -> [16,128] then broadcast rows -> [128, 2048]
            c_row = crpool.tile([128, S], dt.float32, tag="cr")
            ps_t = misc_ps.tile([128, 512], dt.float32, tag="m")
            nc.tensor.transpose(ps_t[0:16, 0:128], cs[:, 0:NKB], ident_t[:])
            csT = cspool.tile([16, 128], dt.float32, tag="csT")
            nc.vector.tensor_copy(csT[:], ps_t[0:16, 0:128])
            cs_flat = lspool.tile([1, S], dt.float32, tag="lsf")
            nc.sync.dma_start(cs_flat[0:1, :], csT[:])
            for ch in range(4):
                ps_c = misc_ps.tile([128, 512], dt.float32, tag="m")
                nc.tensor.matmul(ps_c[:], ones_1x128[:],
                                 cs_flat[0:1, ch * 512:(ch + 1) * 512],
                                 start=True, stop=True)
                nc.vector.tensor_copy(c_row[:, ch * 512:(ch + 1) * 512], ps_c[:])

            # vectorized bisection: 128 thresholds/partition per phase
            lo = smpool.tile([128, 1], dt.float32, tag="s1")
            nc.vector.memset(lo[:], 0.0)
            step = smpool.tile([128, 1], dt.float32, tag="s1")
            nc.vector.tensor_reduce(step[:], c_row[:], axis=AX.X, op=A.max)
            nc.vector.tensor_scalar(out=step[:], in0=step[:], scalar1=1.0 / 129.0,
                                    scalar2=None, op0=A.mult)
            for ph in range(NPH):
                T = smpool.tile([128, 1], dt.float32, tag="s1")
                nc.vector.tensor_scalar(out=T[:], in0=iota_t[:],
                                        scalar1=step[:, 0:1], scalar2=lo[:, 0:1],
                                        op0=A.mult, op1=A.add)
                scb = scrpool.tile([128, S], dt.float16, tag="sc16")
                cnt = smpool.tile([128, 1], dt.float32, tag="s1")
                nc.vector.tensor_scalar(out=scb[:], in0=c_row[:],
                                        scalar1=T[:, 0:1], scalar2=None,
                                        op0=A.is_gt, op1=A.add, accum_out=cnt[:])
                ge = smpool.tile([128, 1], dt.float32, tag="s1")
                nc.vector.tensor_scalar(out=ge[:], in0=cnt[:], scalar1=NDROP_THR,
                                        scalar2=None, op0=A.is_gt)
                m_t = smpool.tile([128, 1], dt.float32, tag="s1")
                nc.gpsimd.partition_all_reduce(m_t[:], ge[:], channels=128,
                                               reduce_op=bass_isa.ReduceOp.add)
                lo2 = smpool.tile([128, 1], dt.float32, tag="s1")
                nc.vector.tensor_scalar(out=lo2[:], in0=m_t[:],
                                        scalar1=step[:, 0:1], scalar2=lo[:, 0:1],
                                        op0=A.mult, op1=A.add)
                lo = lo2
                step2 = smpool.tile([128, 1], dt.float32, tag="s1")
                nc.vector.tensor_scalar(out=step2[:], in0=step[:],
                                        scalar1=1.0 / 129.0, scalar2=None,
                                        op0=A.mult)
                step = step2
            thr = smpool.tile([128, 1], dt.float32, tag="s1")
            nc.vector.tensor_scalar(out=thr[:], in0=step[:], scalar1=64.5,
                                    scalar2=lo[:, 0:1], op0=A.mult, op1=A.add)

            m_keep = cspool.tile([128, NKB], dt.float32, tag="mk")
            nc.vector.tensor_scalar(out=m_keep[:], in0=cs[:, 0:NKB],
                                    scalar1=thr[:, 0:1], scalar2=None, op0=A.is_gt)
            m_drop = cspool.tile([128, NKB], dt.float32, tag="md")
            nc.vector.tensor_scalar(out=m_drop[:], in0=m_keep[:], scalar1=-1.0,
                                    scalar2=1.0, op0=A.mult, op1=A.add)

            # complement AV2 over dropped columns; O = (O_full - C) * r
            vd = vdpool.tile([128, NKB, 64], dt.float16, tag="vd", name=f"vd{h}")
            for kb in range(NKB):
                nc.vector.tensor_scalar(out=vd[:, kb, :], in0=v_aug[h][:, kb, 0:64],
                                        scalar1=m_drop[:, kb:kb + 1], scalar2=None,
                                        op0=A.mult)
            tile_idx, row0 = h // 2, (h % 2) * 64
            for qb in range(NQ):
                qs = slice(qb * 512, (qb + 1) * 512)
                cps = misc_ps.tile([128, 512], dt.float32, tag="m")
                for kb in range(NKB):
                    nc.tensor.matmul(cps[0:64, :], vd[:, kb, :],
                                     e_t[kb][:, qs],
                                     start=(kb == 0), stop=(kb == NKB - 1))
                t1 = scrpool.tile([64, 512], dt.float32, tag="t1")
                nc.vector.tensor_tensor(out=t1[:], in0=o_full[0:64, qs],
                                        in1=cps[0:64, :], op=A.subtract)
                if row0 == 0:
                    nc.vector.tensor_tensor(out=ocat[tile_idx][0:64, qs],
                                            in0=t1[:], in1=rb[0:64, qs], op=A.mult)
                else:
                    t2 = scrpool.tile([64, 512], dt.float16, tag="t2")
                    nc.vector.tensor_tensor(out=t2[:], in0=t1[:],
                                            in1=rb[0:64, qs], op=A.mult)
                    nc.sync.dma_start(ocat[tile_idx][64:128, qs], t2[:])

        # ---------------- Phase O: out-projection partial ----------------
        with tc.tile_pool(name="oW", bufs=1) as wopool, \
             tc.tile_pool(name="oS", bufs=2) as ospool:
            wo_t = wopool.tile([128, 2 * DM], dt.float16)
            for ct in range(2):
                nc.sync.dma_start(wo_t[:, ct * DM:(ct + 1) * DM],
                                  wo[ct * 128:(ct + 1) * 128, :])
            for ot in range(DM // 128):
                for qb in range(NQ):
                    pso = misc_ps.tile([128, 512], dt.float32, tag="m")
                    for ct in range(2):
                        nc.tensor.matmul(
                            pso[:],
                            wo_t[:, ct * DM + ot * 128: ct * DM + ot * 128 + 128],
                            ocat[ct][:, qb * 512:(qb + 1) * 512],
                            start=(ct == 0), stop=(ct == 1))
                    osb = ospool.tile([128, 512], dt.float32, tag="osb")
                    nc.vector.tensor_copy(osb[:], pso[:])
                    nc.sync.dma_start(out_part[ot * 128:(ot + 1) * 128,
                                               qb * 512:(qb + 1) * 512], osb[:])
    nc.compile()
    return nc


def _get_nc():
    if "nc" not in _CACHE:
        nc = bacc_mod.Bacc('TRN2', target_bir_lowering=False)
        _emit(nc)
        _CACHE["nc"] = nc
    return _CACHE["nc"]


def _split16(x):
    hi = x.astype(np.float16)
    lo = (x - hi.astype(np.float32)).astype(np.float16)
    return hi, lo


def kernel(q, k, v, Wq, bq, Wk, bk, Wv, bv, Wo, bo):
    q, k, v = (np.asarray(a, np.float32) for a in (q, k, v))
    Wq, bq, Wk, bk, Wv, bv, Wo, bo = (np.asarray(a, np.float32) for a in
                                      (Wq, bq, Wk, bk, Wv, bv, Wo, bo))
    nc = _get_nc()

    xt = {}
    for b in range(B):
        xq16 = np.ascontiguousarray(q[b].T).astype(np.float16)
        kh, kl = _split16(np.ascontiguousarray(k[b].T))
        xv16 = np.ascontiguousarray(v[b].T).astype(np.float16)
        xt[b] = (xq16, kh, kl, xv16)

    iota1 = np.arange(1, 129, dtype=np.float32).reshape(128, 1)
    ident = np.eye(128, dtype=np.float32)

    in_maps = []
    for core in range(N_CORES):
        b = core // 4
        h0 = (core % 4) * HPC
        cols = slice(h0 * DK, (h0 + HPC) * DK)
        xq16, kh, kl, xv16 = xt[b]
        wkh_, wkl_ = _split16(np.ascontiguousarray(Wk[cols].T))
        in_maps.append({
            "xq": xq16, "xkh": kh, "xkl": kl, "xv": xv16,
            "wq": np.ascontiguousarray(Wq[cols].T).astype(np.float16),
            "wkh": wkh_, "wkl": wkl_,
            "wv": np.ascontiguousarray(Wv[cols].T).astype(np.float16),
            "wo": np.ascontiguousarray(Wo[:, cols].T).astype(np.float16),
            "bqi": np.ascontiguousarray(bq[cols].reshape(2, 128).T),
            "bki": np.ascontiguousarray(bk[cols].reshape(2, 128).T),
            "bvi": np.ascontiguousarray(bv[cols].reshape(1, CPC)),
            "iota1": iota1, "ident": ident,
        })

    from concourse.bass_utils import run_bass_kernel_spmd
    _CACHE["last_in_maps"] = in_maps
    res = run_bass_kernel_spmd(nc, in_maps, core_ids=list(range(N_CORES)))
    _CACHE["last_res"] = res

    out = np.zeros((B, S, DM), np.float32)
    for core in range(N_CORES):
        b = core // 4
        out[b] += res.results[core]["out_part"].T
    out += bo.reshape(1, 1, DM)
    return out


# revision 13
# speedup vs baseline: 1.3269x; 1.0753x over previous
"""Sparse (top-k pruned) multi-head attention on 8 Trainium2 NeuronCores.

Sharding: batch(2) x head-groups(4 heads) -> 8 cores.

Per core (4 heads h=0..3, pairs ct=h//2):
  P: q-proj 1-term fp16; k-proj 3-term fp16 hi/lo (precision carrier);
     v-proj 1-term fp16 -> v_aug [128,16,65] (ones col).
     Scores operands: kstack_h = [khi;klo] dims stacked, qdup_h = [q;q].
  A (per head): scores S^T = kstack^T qdup (one 128-contraction fp16 mm
     per (kb,qs)); ACT exp (scale=1/8) -> E fp16 + f32 accum csA (exact
     unnormalized colsum); AV1 = E @ [v;1] -> O_full + row sums l;
     r = approx_recip(broadcast l); colsum = rbar*csA + sum(E*(r-rbar))
     (compensated, near-f32); C_row = PE-transpose+broadcast of colsums;
     128-thresholds-per-partition bisection x4 phases -> exact top-1843
     threshold; complement AV2 = E @ (v*drop); O = (O_full - C)*r.
  O: out-projection partial (tensor-parallel on d_model), f32 partials.
Host sums per-core partials and adds bo.
"""
import sys
for p in ('/opt/trn_rl_repo', '/opt/pypackages'):
    if p not in sys.path:
        sys.path.insert(0, p)
import numpy as np
from contextlib import ExitStack

import concourse.bass as bass
import concourse.bacc as bacc_mod
import concourse.bass_isa as bass_isa
import concourse.tile as tile
import concourse.mybir as mybir
from concourse import library_config

dt = mybir.dt
F = mybir.ActivationFunctionType
A = mybir.AluOpType
AX = mybir.AxisListType

B, S, DM, H, DK = 2, 2048, 1024, 16, 64
HPC = 4              # heads per core
CPC = HPC * DK       # 256 proj columns per core
KEEP = int(S * 0.9)  # 1843
N_CORES = 8
KT = DM // 128       # 8 contraction tiles for projections
NQ = S // 512        # 4 query chunks
NKB = S // 128       # 16 key tiles
NPH = 4              # bisection phases (129-way each)
NDROP_THR = float(KEEP) - 0.5   # count(c > t) target boundary: 1842.5

_CACHE = {}


def _emit(nc):
    ei = lambda n, s, d: nc.dram_tensor(n, s, d, kind="ExternalInput")
    xq = ei("xq", [DM, S], dt.float16)
    xkh = ei("xkh", [DM, S], dt.float16)
    xkl = ei("xkl", [DM, S], dt.float16)
    xv = ei("xv", [DM, S], dt.float16)
    wq = ei("wq", [DM, CPC], dt.float16)
    wkh = ei("wkh", [DM, CPC], dt.float16)
    wkl = ei("wkl", [DM, CPC], dt.float16)
    wv = ei("wv", [DM, CPC], dt.float16)
    wo = ei("wo", [CPC, DM], dt.float16)
    bqi = ei("bqi", [128, 2], dt.float32)
    bki = ei("bki", [128, 2], dt.float32)
    bvi = ei("bvi", [1, CPC], dt.float32)
    iota_in = ei("iota1", [128, 1], dt.float32)   # values 1..128
    ident_in = ei("ident", [128, 128], dt.float32)
    out_part = nc.dram_tensor("out_part", [DM, S], dt.float32, kind="ExternalOutput")

    with tile.TileContext(nc) as tc, ExitStack() as ctx:
        nc.gpsimd.load_library(library_config.attn)

        # ---------- long-lived pools ----------
        cpool = ctx.enter_context(tc.tile_pool(name="const", bufs=1))
        qkpool = ctx.enter_context(tc.tile_pool(name="qk", bufs=1))
        vpool = ctx.enter_context(tc.tile_pool(name="vaug", bufs=1))
        ocpool = ctx.enter_context(tc.tile_pool(name="ocat", bufs=1))
        # psum pools (total <= 8 banks): scores 2x2 + av1 2x1 + misc 2x1
        misc_ps = ctx.enter_context(tc.tile_pool(name="mps", bufs=2, space="PSUM"))

        ones_1x128 = cpool.tile([1, 128], dt.float32, tag="c1")
        nc.vector.memset(ones_1x128[:], 1.0)
        iota_t = cpool.tile([128, 1], dt.float32, tag="c2")
        nc.sync.dma_start(iota_t[:], iota_in[:, :])
        ident_t = cpool.tile([128, 128], dt.float32, tag="c3")
        nc.sync.dma_start(ident_t[:], ident_in[:, :])
        bq_sb = cpool.tile([128, 2], dt.float32, tag="c4")
        nc.sync.dma_start(bq_sb[:], bqi[:, :])
        bk_sb = cpool.tile([128, 2], dt.float32, tag="c5")
        nc.sync.dma_start(bk_sb[:], bki[:, :])

        qdup = [qkpool.tile([128, S], dt.float16, tag=f"qd{h}", name=f"qd{h}")
                for h in range(HPC)]
        kstack = [qkpool.tile([128, S], dt.float16, tag=f"ks{h}", name=f"ks{h}")
                  for h in range(HPC)]
        v_aug = [vpool.tile([128, NKB, 65], dt.float16, tag=f"va{h}", name=f"va{h}")
                 for h in range(HPC)]
        ocat = [ocpool.tile([128, S], dt.float16, tag=f"oc{i}", name=f"oc{i}")
                for i in range(2)]

        # ---------------- Phase P: projections ----------------
        with tc.tile_pool(name="pW", bufs=1) as wpool, \
             tc.tile_pool(name="pX", bufs=8) as xpool, \
             tc.tile_pool(name="pT", bufs=2) as tpool, \
             tc.tile_pool(name="pPs", bufs=4, space="PSUM") as pj_ps:

            wq_t = wpool.tile([128, KT * CPC], dt.float16)
            wkh_t = wpool.tile([128, KT * CPC], dt.float16)
            wkl_t = wpool.tile([128, KT * CPC], dt.float16)
            wv_t = wpool.tile([128, KT * CPC], dt.float16)
            for kt in range(KT):
                sl = slice(kt * CPC, (kt + 1) * CPC)
                rows = slice(kt * 128, (kt + 1) * 128)
                nc.sync.dma_start(wq_t[:, sl], wq[rows, :])
                nc.sync.dma_start(wkh_t[:, sl], wkh[rows, :])
                nc.sync.dma_start(wkl_t[:, sl], wkl[rows, :])
                nc.sync.dma_start(wv_t[:, sl], wv[rows, :])
            bv_sb = wpool.tile([1, CPC], dt.float32)
            nc.sync.dma_start(bv_sb[:], bvi[:, :])
            ps_bv = misc_ps.tile([128, 512], dt.float32, tag="m")
            nc.tensor.matmul(ps_bv[:, 0:CPC], ones_1x128[:], bv_sb[:],
                             start=True, stop=True)
            bv_bc = wpool.tile([128, CPC], dt.float32)
            nc.vector.tensor_copy(bv_bc[:], ps_bv[:, 0:CPC])

            # ---- q projection (1-term) + k projection (3-term hi/lo) ----
            xq_t = []
            xkh_t = []
            xkl_t = []
            for kt in range(KT):
                rows = slice(kt * 128, (kt + 1) * 128)
                t1 = xpool.tile([128, S], dt.float16, tag="xq", name=f"xq{kt}")
                nc.sync.dma_start(t1[:], xq[rows, :])
                xq_t.append(t1)
                t2 = xpool.tile([128, S], dt.float16, tag="xkh", name=f"xkh{kt}")
                nc.sync.dma_start(t2[:], xkh[rows, :])
                xkh_t.append(t2)
                t3 = xpool.tile([128, S], dt.float16, tag="xkl", name=f"xkl{kt}")
                nc.sync.dma_start(t3[:], xkl[rows, :])
                xkl_t.append(t3)
            for nt in range(NQ):
                qs = slice(nt * 512, (nt + 1) * 512)
                for ct in range(2):
                    hA, hB = 2 * ct, 2 * ct + 1
                    # q: single term
                    psq = pj_ps.tile([128, 512], dt.float32, tag="pj")
                    for kt in range(KT):
                        wsl = slice(kt * CPC + ct * 128, kt * CPC + ct * 128 + 128)
                        nc.tensor.matmul(psq[:], wq_t[:, wsl], xq_t[kt][:, qs],
                                         start=(kt == 0), stop=(kt == KT - 1))
                    # qdup: rows 0:64 then DMA-duplicate to rows 64:128
                    nc.vector.tensor_scalar(
                        out=qdup[hA][0:64, qs], in0=psq[0:64, :],
                        scalar1=bq_sb[0:64, ct:ct + 1], scalar2=None, op0=A.add)
                    nc.vector.tensor_scalar(
                        out=qdup[hB][0:64, qs], in0=psq[64:128, :],
                        scalar1=bq_sb[64:128, ct:ct + 1], scalar2=None, op0=A.add)
                    nc.sync.dma_start(qdup[hA][64:128, qs], qdup[hA][0:64, qs])
                    nc.sync.dma_start(qdup[hB][64:128, qs], qdup[hB][0:64, qs])

                    # k: 3 terms -> f32 psum
                    psk = pj_ps.tile([128, 512], dt.float32, tag="pj")
                    i_mm = 0
                    for kt in range(KT):
                        wsl = slice(kt * CPC + ct * 128, kt * CPC + ct * 128 + 128)
                        for (wt, xt) in ((wkh_t, xkh_t[kt]), (wkh_t, xkl_t[kt]),
                                         (wkl_t, xkh_t[kt])):
                            nc.tensor.matmul(psk[:], wt[:, wsl], xt[:, qs],
                                             start=(i_mm == 0),
                                             stop=(i_mm == 3 * KT - 1))
                            i_mm += 1
                    # kA: hi aligned rows 0:64; lo via tmp + shift-DMA
                    nc.vector.tensor_scalar(
                        out=kstack[hA][0:64, qs], in0=psk[0:64, :],
                        scalar1=bk_sb[0:64, ct:ct + 1], scalar2=None, op0=A.add)
                    tfA = tpool.tile([128, 512], dt.float32, tag="tf")
                    nc.vector.tensor_scalar(
                        out=tfA[0:64, :], in0=psk[0:64, :],
                        scalar1=bk_sb[0:64, ct:ct + 1], scalar2=None, op0=A.add)
                    tA16 = tpool.tile([128, 512], dt.float16, tag="t16")
                    nc.vector.tensor_tensor(out=tA16[0:64, :], in0=tfA[0:64, :],
                                            in1=kstack[hA][0:64, qs], op=A.subtract)
                    nc.sync.dma_start(kstack[hA][64:128, qs], tA16[0:64, :])
                    # kB: hi into tmp rows 64:128 (aligned), shift-DMA to rows 0:64;
                    #     lo aligned rows 64:128
                    tB16 = tpool.tile([128, 512], dt.float16, tag="t16")
                    nc.vector.tensor_scalar(
                        out=tB16[64:128, :], in0=psk[64:128, :],
                        scalar1=bk_sb[64:128, ct:ct + 1], scalar2=None, op0=A.add)
                    nc.sync.dma_start(kstack[hB][0:64, qs], tB16[64:128, :])
                    tfB = tpool.tile([128, 512], dt.float32, tag="tf")
                    nc.vector.tensor_scalar(
                        out=tfB[64:128, :], in0=psk[64:128, :],
                        scalar1=bk_sb[64:128, ct:ct + 1], scalar2=None, op0=A.add)
                    nc.vector.tensor_tensor(out=kstack[hB][64:128, qs],
                                            in0=tfB[64:128, :], in1=tB16[64:128, :],
                                            op=A.subtract)

            # ---- v projection (1-term fp16) ----
            xv_t = []
            for kt in range(KT):
                rows = slice(kt * 128, (kt + 1) * 128)
                t4 = xpool.tile([128, S], dt.float16, tag="xv", name=f"xv{kt}")
                nc.sync.dma_start(t4[:], xv[rows, :])
                xv_t.append(t4)
            for tb in range(NKB):
                tsl = slice(tb * 128, (tb + 1) * 128)
                psv = pj_ps.tile([128, 512], dt.float32, tag="pj")
                for kt in range(KT):
                    nc.tensor.matmul(psv[:, 0:CPC], xv_t[kt][:, tsl],
                                     wv_t[:, kt * CPC:(kt + 1) * CPC],
                                     start=(kt == 0), stop=(kt == KT - 1))
                for h in range(HPC):
                    nc.vector.tensor_tensor(
                        out=v_aug[h][:, tb, 0:64], in0=psv[:, h * 64:(h + 1) * 64],
                        in1=bv_bc[:, h * 64:(h + 1) * 64], op=A.add)
            for h in range(HPC):
                nc.vector.memset(v_aug[h][:, :, 64:65], 1.0)

        # ---------------- Phase A: per-head attention ----------------
        score_ps = ctx.enter_context(tc.tile_pool(name="scps", bufs=2, space="PSUM"))
        av1_ps = ctx.enter_context(tc.tile_pool(name="avps", bufs=2, space="PSUM"))
        epool = ctx.enter_context(tc.tile_pool(name="E", bufs=16))
        ofpool = ctx.enter_context(tc.tile_pool(name="ofull", bufs=2))
        rbpool = ctx.enter_context(tc.tile_pool(name="rb", bufs=2))
        w16pool = ctx.enter_context(tc.tile_pool(name="w16", bufs=2))
        crpool = ctx.enter_context(tc.tile_pool(name="crow", bufs=1))
        scrpool = ctx.enter_context(tc.tile_pool(name="scr", bufs=2))
        cspool = ctx.enter_context(tc.tile_pool(name="cs", bufs=2))
        lspool = ctx.enter_context(tc.tile_pool(name="ls", bufs=2))
        smpool = ctx.enter_context(tc.tile_pool(name="sm", bufs=4))
        vdpool = ctx.enter_context(tc.tile_pool(name="vd", bufs=2))
        for h in range(HPC):
            e_t = []
            csA2 = cspool.tile([128, 2 * NKB], dt.float32, tag="csA2")
            av1 = [av1_ps.tile([65, 512], dt.float32, tag="av", name=f"av{h}_{qb}")
                   for qb in range(2)]
            for kb in range(NKB):
                et = epool.tile([128, S], dt.float16, tag="E", name=f"E{h}_{kb}")
                e_t.append(et)
                kcols = slice(kb * 128, (kb + 1) * 128)
                for half in range(2):
                    sc = score_ps.tile([128, 1024], dt.float32, tag="sc")
                    for qq in range(2):
                        qs = slice((half * 2 + qq) * 512, (half * 2 + qq) * 512 + 512)
                        nc.tensor.matmul(sc[:, qq * 512:(qq + 1) * 512],
                                         kstack[h][:, kcols], qdup[h][:, qs],
                                         start=True, stop=True)
                    nc.scalar.activation(
                        et[:, half * 1024:(half + 1) * 1024], sc[:], F.Exp,
                        bias=0.0, scale=0.125,
                        accum_out=csA2[:, 2 * kb + half:2 * kb + half + 1])
                # AV1 for qb 0,1 accumulates during the scores pass
                for qb in range(2):
                    nc.tensor.matmul(av1[qb][:], v_aug[h][:, kb, :],
                                     e_t[kb][:, qb * 512:(qb + 1) * 512],
                                     start=(kb == 0), stop=(kb == NKB - 1))

            # drain av1 qb 0,1 -> o_full, then run qb 2,3
            o_full = ofpool.tile([65, S], dt.float32, tag="of", name=f"of{h}")
            for qb in range(2):
                nc.vector.tensor_copy(o_full[:, qb * 512:(qb + 1) * 512], av1[qb][:])
            av2r = [av1_ps.tile([65, 512], dt.float32, tag="av", name=f"av2_{h}_{qb}")
                    for qb in range(2)]
            for kb in range(NKB):
                for qb in range(2):
                    nc.tensor.matmul(av2r[qb][:], v_aug[h][:, kb, :],
                                     e_t[kb][:, (2 + qb) * 512:(3 + qb) * 512],
                                     start=(kb == 0), stop=(kb == NKB - 1))
            for qb in range(2):
                nc.vector.tensor_copy(o_full[:, (2 + qb) * 512:(3 + qb) * 512],
                                      av2r[qb][:])

            # rb = approx 1/l broadcast to 128 partitions; rbar; w16 = r - rbar
            rb = rbpool.tile([128, S], dt.float32, tag="rb", name=f"rb{h}")
            l_sb = lspool.tile([1, S], dt.float32, tag="lsf")
            nc.sync.dma_start(l_sb[:], o_full[64:65, :])
            r_sb = lspool.tile([1, S], dt.float32, tag="lsf")
            nc.vector.reciprocal(r_sb[:], l_sb[:])
            for qb in range(NQ):
                qs = slice(qb * 512, (qb + 1) * 512)
                ps_l = misc_ps.tile([128, 512], dt.float32, tag="m")
                nc.tensor.matmul(ps_l[:], ones_1x128[:], r_sb[0:1, qs],
                                 start=True, stop=True)
                nc.vector.tensor_copy(rb[:, qs], ps_l[:])
            rmax = smpool.tile([128, 1], dt.float32, tag="s1")
            nc.vector.tensor_reduce(rmax[:], rb[:], axis=AX.X, op=A.max)
            rbar = smpool.tile([128, 1], dt.float32, tag="s1")
            nc.vector.tensor_scalar(out=rbar[:], in0=rmax[:], scalar1=0.97,
                                    scalar2=None, op0=A.mult)
            w16 = w16pool.tile([128, S], dt.float16, tag="w16", name=f"w16{h}")
            nc.vector.tensor_scalar(out=w16[:], in0=rb[:], scalar1=rbar[:, 0:1],
                                    scalar2=None, op0=A.subtract)

            # compensated colsum: cs = rbar*csA + sum_q E*(r-rbar)
            csB = cspool.tile([128, NKB], dt.float32, tag="csB")
            for kb in range(NKB):
                scr = scrpool.tile([128, S], dt.float16, tag="sc16")
                nc.vector.tensor_tensor(out=scr[:], in0=e_t[kb][:], in1=w16[:],
                                        op=A.mult)
                nc.vector.reduce_sum(csB[:, kb:kb + 1], scr[:], axis=AX.X)
            cs = cspool.tile([128, 32], dt.float32, tag="cs")
            nc.vector.tensor_tensor(out=cs[:, 0:NKB],
                                    in0=csA2[:, 0:2 * NKB:2],
                                    in1=csA2[:, 1:2 * NKB:2], op=A.add)
            nc.vector.tensor_scalar(out=cs[:, 0:NKB], in0=cs[:, 0:NKB],
                                    scalar1=rbar[:, 0:1], scalar2=None, op0=A.mult)
            nc.vector.tensor_tensor(out=cs[:, 0:NKB], in0=cs[:, 0:NKB],
                                    in1=csB[:], op=A.add)

            # C_row: transpose cs -> [16,128] then broadcast rows -> [128, 2048]
            c_row = crpool.tile([128, S], dt.float32, tag="cr")
            ps_t = misc_ps.tile([128, 512], dt.float32, tag="m")
            nc.tensor.transpose(ps_t[0:16, 0:128], cs[:, 0:NKB], ident_t[:])
            csT = cspool.tile([16, 128], dt.float32, tag="csT")
            nc.vector.tensor_copy(csT[:], ps_t[0:16, 0:128])
            cs_flat = lspool.tile([1, S], dt.float32, tag="lsf")
            nc.sync.dma_start(cs_flat[0:1, :], csT[:])
            for ch in range(4):
                ps_c = misc_ps.tile([128, 512], dt.float32, tag="m")
                nc.tensor.matmul(ps_c[:], ones_1x128[:],
                                 cs_flat[0:1, ch * 512:(ch + 1) * 512],
                                 start=True, stop=True)
                nc.vector.tensor_copy(c_row[:, ch * 512:(ch + 1) * 512], ps_c[:])

            # vectorized bisection: 128 thresholds/partition per phase
            lo = smpool.tile([128, 1], dt.float32, tag="s1")
            nc.vector.memset(lo[:], 0.0)
            step = smpool.tile([128, 1], dt.float32, tag="s1")
            nc.vector.tensor_reduce(step[:], c_row[:], axis=AX.X, op=A.max)
            nc.vector.tensor_scalar(out=step[:], in0=step[:], scalar1=1.0 / 129.0,
                                    scalar2=None, op0=A.mult)
            for ph in range(NPH):
                T = smpool.tile([128, 1], dt.float32, tag="s1")
                nc.vector.tensor_scalar(out=T[:], in0=iota_t[:],
                                        scalar1=step[:, 0:1], scalar2=lo[:, 0:1],
                                        op0=A.mult, op1=A.add)
                scb = scrpool.tile([128, S], dt.float16, tag="sc16")
                cnt = smpool.tile([128, 1], dt.float32, tag="s1")
                nc.vector.tensor_scalar(out=scb[:], in0=c_row[:],
                                        scalar1=T[:, 0:1], scalar2=None,
                                        op0=A.is_gt, op1=A.add, accum_out=cnt[:])
                ge = smpool.tile([128, 1], dt.float32, tag="s1")
                nc.vector.tensor_scalar(out=ge[:], in0=cnt[:], scalar1=NDROP_THR,
                                        scalar2=None, op0=A.is_gt)
                m_t = smpool.tile([128, 1], dt.float32, tag="s1")
                nc.gpsimd.partition_all_reduce(m_t[:], ge[:], channels=128,
                                               reduce_op=bass_isa.ReduceOp.add)
                lo2 = smpool.tile([128, 1], dt.float32, tag="s1")
                nc.vector.tensor_scalar(out=lo2[:], in0=m_t[:],
                                        scalar1=step[:, 0:1], scalar2=lo[:, 0:1],
                                        op0=A.mult, op1=A.add)
                lo = lo2
                step2 = smpool.tile([128, 1], dt.float32, tag="s1")
                nc.vector.tensor_scalar(out=step2[:], in0=step[:],
                                        scalar1=1.0 / 129.0, scalar2=None,
                                        op0=A.mult)
                step = step2
            thr = smpool.tile([128, 1], dt.float32, tag="s1")
            nc.vector.tensor_scalar(out=thr[:], in0=step[:], scalar1=64.5,
                                    scalar2=lo[:, 0:1], op0=A.mult, op1=A.add)

            m_keep = cspool.tile([128, NKB], dt.float32, tag="mk")
            nc.vector.tensor_scalar(out=m_keep[:], in0=cs[:, 0:NKB],
                                    scalar1=thr[:, 0:1], scalar2=None, op0=A.is_gt)
            m_drop = cspool.tile([128, NKB], dt.float32, tag="md")
            nc.vector.tensor_scalar(out=m_drop[:], in0=m_keep[:], scalar1=-1.0,
                                    scalar2=1.0, op0=A.mult, op1=A.add)

            # complement AV2 over dropped columns; O = (O_full - C) * r
            vd = vdpool.tile([128, NKB, 64], dt.float16, tag="vd", name=f"vd{h}")
            for kb in range(NKB):
                nc.vector.tensor_scalar(out=vd[:, kb, :], in0=v_aug[h][:, kb, 0:64],
                                        scalar1=m_drop[:, kb:kb + 1], scalar2=None,
                                        op0=A.mult)
            tile_idx, row0 = h // 2, (h % 2) * 64
            for qb in range(NQ):
                qs = slice(qb * 512, (qb + 1) * 512)
                cps = misc_ps.tile([128, 512], dt.float32, tag="m")
                for kb in range(NKB):
                    nc.tensor.matmul(cps[0:64, :], vd[:, kb, :],
                                     e_t[kb][:, qs],
                                     start=(kb == 0), stop=(kb == NKB - 1))
                t1 = scrpool.tile([64, 512], dt.float32, tag="t1")
                nc.vector.tensor_tensor(out=t1[:], in0=o_full[0:64, qs],
                                        in1=cps[0:64, :], op=A.subtract)
                if row0 == 0:
                    nc.vector.tensor_tensor(out=ocat[tile_idx][0:64, qs],
                                            in0=t1[:], in1=rb[0:64, qs], op=A.mult)
                else:
                    t2 = scrpool.tile([64, 512], dt.float16, tag="t2")
                    nc.vector.tensor_tensor(out=t2[:], in0=t1[:],
                                            in1=rb[0:64, qs], op=A.mult)
                    nc.sync.dma_start(ocat[tile_idx][64:128, qs], t2[:])

        # ---------------- Phase O: out-projection partial ----------------
        with tc.tile_pool(name="oW", bufs=1) as wopool, \
             tc.tile_pool(name="oS", bufs=2) as ospool:
            wo_t = wopool.tile([128, 2 * DM], dt.float16)
            for ct in range(2):
                nc.sync.dma_start(wo_t[:, ct * DM:(ct + 1) * DM],
                                  wo[ct * 128:(ct + 1) * 128, :])
            for ot in range(DM // 128):
                for qb in range(NQ):
                    pso = misc_ps.tile([128, 512], dt.float32, tag="m")
                    for ct in range(2):
                        nc.tensor.matmul(
                            pso[:],
                            wo_t[:, ct * DM + ot * 128: ct * DM + ot * 128 + 128],
                            ocat[ct][:, qb * 512:(qb + 1) * 512],
                            start=(ct == 0), stop=(ct == 1))
                    osb = ospool.tile([128, 512], dt.float32, tag="osb")
                    nc.scalar.copy(osb[:], pso[:])
                    nc.sync.dma_start(out_part[ot * 128:(ot + 1) * 128,
                                               qb * 512:(qb + 1) * 512], osb[:])
    nc.compile()
    return nc


def _get_nc():
    if "nc" not in _CACHE:
        nc = bacc_mod.Bacc('TRN2', target_bir_lowering=False)
        _emit(nc)
        _CACHE["nc"] = nc
    return _CACHE["nc"]


def _split16(x):
    hi = x.astype(np.float16)
    lo = (x - hi.astype(np.float32)).astype(np.float16)
    return hi, lo


def kernel(q, k, v, Wq, bq, Wk, bk, Wv, bv, Wo, bo):
    q, k, v = (np.asarray(a, np.float32) for a in (q, k, v))
    Wq, bq, Wk, bk, Wv, bv, Wo, bo = (np.asarray(a, np.float32) for a in
                                      (Wq, bq, Wk, bk, Wv, bv, Wo, bo))
    nc = _get_nc()

    xt = {}
    for b in range(B):
        xq16 = np.ascontiguousarray(q[b].T).astype(np.float16)
        kh, kl = _split16(np.ascontiguousarray(k[b].T))
        xv16 = np.ascontiguousarray(v[b].T).astype(np.float16)
        xt[b] = (xq16, kh, kl, xv16)

    iota1 = np.arange(1, 129, dtype=np.float32).reshape(128, 1)
    ident = np.eye(128, dtype=np.float32)

    in_maps = []
    for core in range(N_CORES):
        b = core // 4
        h0 = (core % 4) * HPC
        cols = slice(h0 * DK, (h0 + HPC) * DK)
        xq16, kh, kl, xv16 = xt[b]
        wkh_, wkl_ = _split16(np.ascontiguousarray(Wk[cols].T))
        in_maps.append({
            "xq": xq16, "xkh": kh, "xkl": kl, "xv": xv16,
            "wq": np.ascontiguousarray(Wq[cols].T).astype(np.float16),
            "wkh": wkh_, "wkl": wkl_,
            "wv": np.ascontiguousarray(Wv[cols].T).astype(np.float16),
            "wo": np.ascontiguousarray(Wo[:, cols].T).astype(np.float16),
            "bqi": np.ascontiguousarray(bq[cols].reshape(2, 128).T),
            "bki": np.ascontiguousarray(bk[cols].reshape(2, 128).T),
            "bvi": np.ascontiguousarray(bv[cols].reshape(1, CPC)),
            "iota1": iota1, "ident": ident,
        })

    from concourse.bass_utils import run_bass_kernel_spmd
    _CACHE["last_in_maps"] = in_maps
    res = run_bass_kernel_spmd(nc, in_maps, core_ids=list(range(N_CORES)))
    _CACHE["last_res"] = res

    out = np.zeros((B, S, DM), np.float32)
    for core in range(N_CORES):
        b = core // 4
        out[b] += res.results[core]["out_part"].T
    out += bo.reshape(1, 1, DM)
    return out


# revision 14
# speedup vs baseline: 1.3644x; 1.0282x over previous
"""Sparse (top-k pruned) multi-head attention on 8 Trainium2 NeuronCores.

Sharding: batch(2) x head-groups(4 heads) -> 8 cores.

Per core (4 heads h=0..3, pairs ct=h//2):
  P: q-proj 1-term fp16; k-proj 3-term fp16 hi/lo (precision carrier);
     v-proj 1-term fp16 -> v_aug [128,16,65] (ones col).
     Scores operands: kstack_h = [khi;klo] dims stacked, qdup_h = [q;q].
  A (per head): scores S^T = kstack^T qdup (one 128-contraction fp16 mm
     per (kb,qs)); ACT exp (scale=1/8) -> E fp16 + f32 accum csA (exact
     unnormalized colsum); AV1 = E @ [v;1] -> O_full + row sums l;
     r = approx_recip(broadcast l); colsum = rbar*csA + sum(E*(r-rbar))
     (compensated, near-f32); C_row = PE-transpose+broadcast of colsums;
     128-thresholds-per-partition bisection x4 phases -> exact top-1843
     threshold; complement AV2 = E @ (v*drop); O = (O_full - C)*r.
  O: out-projection partial (tensor-parallel on d_model), f32 partials.
Host sums per-core partials and adds bo.
"""
import sys
for p in ('/opt/trn_rl_repo', '/opt/pypackages'):
    if p not in sys.path:
        sys.path.insert(0, p)
import numpy as np
from contextlib import ExitStack

import concourse.bass as bass
import concourse.bacc as bacc_mod
import concourse.bass_isa as bass_isa
import concourse.tile as tile
import concourse.mybir as mybir
from concourse import library_config

dt = mybir.dt
F = mybir.ActivationFunctionType
A = mybir.AluOpType
AX = mybir.AxisListType

B, S, DM, H, DK = 2, 2048, 1024, 16, 64
HPC = 4              # heads per core
CPC = HPC * DK       # 256 proj columns per core
KEEP = int(S * 0.9)  # 1843
N_CORES = 8
KT = DM // 128       # 8 contraction tiles for projections
NQ = S // 512        # 4 query chunks
NKB = S // 128       # 16 key tiles
NPH = 4              # bisection phases (129-way each)
NDROP_THR = float(KEEP) - 0.5   # count(c > t) target boundary: 1842.5

_CACHE = {}


def _emit(nc):
    ei = lambda n, s, d: nc.dram_tensor(n, s, d, kind="ExternalInput")
    xq = ei("xq", [DM, S], dt.float16)
    xkh = ei("xkh", [DM, S], dt.float16)
    xkl = ei("xkl", [DM, S], dt.float16)
    xv = ei("xv", [DM, S], dt.float16)
    wq = ei("wq", [DM, CPC], dt.float16)
    wkh = ei("wkh", [DM, CPC], dt.float16)
    wkl = ei("wkl", [DM, CPC], dt.float16)
    wv = ei("wv", [DM, CPC], dt.float16)
    wo = ei("wo", [CPC, DM], dt.float16)
    bqi = ei("bqi", [128, 2], dt.float32)
    bki = ei("bki", [128, 2], dt.float32)
    bvi = ei("bvi", [1, CPC], dt.float32)
    iota_in = ei("iota1", [128, 1], dt.float32)   # values 1..128
    ident_in = ei("ident", [128, 128], dt.float32)
    out_part = nc.dram_tensor("out_part", [DM, S], dt.float32, kind="ExternalOutput")

    with tile.TileContext(nc) as tc, ExitStack() as ctx:
        nc.gpsimd.load_library(library_config.attn)

        # ---------- long-lived pools ----------
        cpool = ctx.enter_context(tc.tile_pool(name="const", bufs=1))
        qkpool = ctx.enter_context(tc.tile_pool(name="qk", bufs=1))
        vpool = ctx.enter_context(tc.tile_pool(name="vaug", bufs=1))
        ocpool = ctx.enter_context(tc.tile_pool(name="ocat", bufs=1))
        # psum pools (total <= 8 banks): scores 2x2 + av1 2x1 + misc 2x1
        misc_ps = ctx.enter_context(tc.tile_pool(name="mps", bufs=2, space="PSUM"))

        ones_1x128 = cpool.tile([1, 128], dt.float32, tag="c1")
        nc.vector.memset(ones_1x128[:], 1.0)
        iota_t = cpool.tile([128, 1], dt.float32, tag="c2")
        nc.sync.dma_start(iota_t[:], iota_in[:, :])
        ident_t = cpool.tile([128, 128], dt.float32, tag="c3")
        nc.sync.dma_start(ident_t[:], ident_in[:, :])
        bq_sb = cpool.tile([128, 2], dt.float32, tag="c4")
        nc.sync.dma_start(bq_sb[:], bqi[:, :])
        bk_sb = cpool.tile([128, 2], dt.float32, tag="c5")
        nc.sync.dma_start(bk_sb[:], bki[:, :])

        qdup = [qkpool.tile([128, S], dt.float16, tag=f"qd{h}", name=f"qd{h}")
                for h in range(HPC)]
        kstack = [qkpool.tile([128, S], dt.float16, tag=f"ks{h}", name=f"ks{h}")
                  for h in range(HPC)]
        v_aug = [vpool.tile([128, NKB, 65], dt.float16, tag=f"va{h}", name=f"va{h}")
                 for h in range(HPC)]
        ocat = [ocpool.tile([128, S], dt.float16, tag=f"oc{i}", name=f"oc{i}")
                for i in range(2)]

        # ---------------- Phase P: projections ----------------
        with tc.tile_pool(name="pW", bufs=1) as wpool, \
             tc.tile_pool(name="pX", bufs=8) as xpool, \
             tc.tile_pool(name="pT", bufs=2) as tpool, \
             tc.tile_pool(name="pPs", bufs=4, space="PSUM") as pj_ps:

            wq_t = wpool.tile([128, KT * CPC], dt.float16)
            wkh_t = wpool.tile([128, KT * CPC], dt.float16)
            wkl_t = wpool.tile([128, KT * CPC], dt.float16)
            wv_t = wpool.tile([128, KT * CPC], dt.float16)
            for kt in range(KT):
                sl = slice(kt * CPC, (kt + 1) * CPC)
                rows = slice(kt * 128, (kt + 1) * 128)
                nc.sync.dma_start(wq_t[:, sl], wq[rows, :])
                nc.sync.dma_start(wkh_t[:, sl], wkh[rows, :])
                nc.sync.dma_start(wkl_t[:, sl], wkl[rows, :])
                nc.sync.dma_start(wv_t[:, sl], wv[rows, :])
            bv_sb = wpool.tile([1, CPC], dt.float32)
            nc.sync.dma_start(bv_sb[:], bvi[:, :])
            ps_bv = misc_ps.tile([128, 512], dt.float32, tag="m")
            nc.tensor.matmul(ps_bv[:, 0:CPC], ones_1x128[:], bv_sb[:],
                             start=True, stop=True)
            bv_bc = wpool.tile([128, CPC], dt.float32)
            nc.vector.tensor_copy(bv_bc[:], ps_bv[:, 0:CPC])

            # ---- q projection (1-term) + k projection (3-term hi/lo) ----
            xq_t = []
            xkh_t = []
            xkl_t = []
            for kt in range(KT):
                rows = slice(kt * 128, (kt + 1) * 128)
                t1 = xpool.tile([128, S], dt.float16, tag="xq", name=f"xq{kt}")
                nc.sync.dma_start(t1[:], xq[rows, :])
                xq_t.append(t1)
                t2 = xpool.tile([128, S], dt.float16, tag="xkh", name=f"xkh{kt}")
                nc.sync.dma_start(t2[:], xkh[rows, :])
                xkh_t.append(t2)
                t3 = xpool.tile([128, S], dt.float16, tag="xkl", name=f"xkl{kt}")
                nc.sync.dma_start(t3[:], xkl[rows, :])
                xkl_t.append(t3)
            for nt in range(NQ):
                qs = slice(nt * 512, (nt + 1) * 512)
                for ct in range(2):
                    hA, hB = 2 * ct, 2 * ct + 1
                    # q: single term
                    psq = pj_ps.tile([128, 512], dt.float32, tag="pj")
                    for kt in range(KT):
                        wsl = slice(kt * CPC + ct * 128, kt * CPC + ct * 128 + 128)
                        nc.tensor.matmul(psq[:], wq_t[:, wsl], xq_t[kt][:, qs],
                                         start=(kt == 0), stop=(kt == KT - 1))
                    # qdup: rows 0:64 then DMA-duplicate to rows 64:128
                    nc.vector.tensor_scalar(
                        out=qdup[hA][0:64, qs], in0=psq[0:64, :],
                        scalar1=bq_sb[0:64, ct:ct + 1], scalar2=None, op0=A.add)
                    nc.vector.tensor_scalar(
                        out=qdup[hB][0:64, qs], in0=psq[64:128, :],
                        scalar1=bq_sb[64:128, ct:ct + 1], scalar2=None, op0=A.add)
                    nc.sync.dma_start(qdup[hA][64:128, qs], qdup[hA][0:64, qs])
                    nc.sync.dma_start(qdup[hB][64:128, qs], qdup[hB][0:64, qs])

                    # k: 3 terms -> f32 psum
                    psk = pj_ps.tile([128, 512], dt.float32, tag="pj")
                    i_mm = 0
                    for kt in range(KT):
                        wsl = slice(kt * CPC + ct * 128, kt * CPC + ct * 128 + 128)
                        for (wt, xt) in ((wkh_t, xkh_t[kt]), (wkh_t, xkl_t[kt]),
                                         (wkl_t, xkh_t[kt])):
                            nc.tensor.matmul(psk[:], wt[:, wsl], xt[:, qs],
                                             start=(i_mm == 0),
                                             stop=(i_mm == 3 * KT - 1))
                            i_mm += 1
                    # kA: hi aligned rows 0:64; lo via tmp + shift-DMA
                    nc.vector.tensor_scalar(
                        out=kstack[hA][0:64, qs], in0=psk[0:64, :],
                        scalar1=bk_sb[0:64, ct:ct + 1], scalar2=None, op0=A.add)
                    tfA = tpool.tile([128, 512], dt.float32, tag="tf")
                    nc.vector.tensor_scalar(
                        out=tfA[0:64, :], in0=psk[0:64, :],
                        scalar1=bk_sb[0:64, ct:ct + 1], scalar2=None, op0=A.add)
                    tA16 = tpool.tile([128, 512], dt.float16, tag="t16")
                    nc.vector.tensor_tensor(out=tA16[0:64, :], in0=tfA[0:64, :],
                                            in1=kstack[hA][0:64, qs], op=A.subtract)
                    nc.sync.dma_start(kstack[hA][64:128, qs], tA16[0:64, :])
                    # kB: hi into tmp rows 64:128 (aligned), shift-DMA to rows 0:64;
                    #     lo aligned rows 64:128
                    tB16 = tpool.tile([128, 512], dt.float16, tag="t16")
                    nc.vector.tensor_scalar(
                        out=tB16[64:128, :], in0=psk[64:128, :],
                        scalar1=bk_sb[64:128, ct:ct + 1], scalar2=None, op0=A.add)
                    nc.sync.dma_start(kstack[hB][0:64, qs], tB16[64:128, :])
                    tfB = tpool.tile([128, 512], dt.float32, tag="tf")
                    nc.vector.tensor_scalar(
                        out=tfB[64:128, :], in0=psk[64:128, :],
                        scalar1=bk_sb[64:128, ct:ct + 1], scalar2=None, op0=A.add)
                    nc.vector.tensor_tensor(out=kstack[hB][64:128, qs],
                                            in0=tfB[64:128, :], in1=tB16[64:128, :],
                                            op=A.subtract)

            # ---- v projection (1-term fp16) ----
            xv_t = []
            for kt in range(KT):
                rows = slice(kt * 128, (kt + 1) * 128)
                t4 = xpool.tile([128, S], dt.float16, tag="xv", name=f"xv{kt}")
                nc.sync.dma_start(t4[:], xv[rows, :])
                xv_t.append(t4)
            for tb in range(NKB):
                tsl = slice(tb * 128, (tb + 1) * 128)
                psv = pj_ps.tile([128, 512], dt.float32, tag="pj")
                for kt in range(KT):
                    nc.tensor.matmul(psv[:, 0:CPC], xv_t[kt][:, tsl],
                                     wv_t[:, kt * CPC:(kt + 1) * CPC],
                                     start=(kt == 0), stop=(kt == KT - 1))
                for h in range(HPC):
                    nc.vector.tensor_tensor(
                        out=v_aug[h][:, tb, 0:64], in0=psv[:, h * 64:(h + 1) * 64],
                        in1=bv_bc[:, h * 64:(h + 1) * 64], op=A.add)
            for h in range(HPC):
                nc.vector.memset(v_aug[h][:, :, 64:65], 1.0)

        # ---------------- Phase A: per-head attention ----------------
        score_ps = ctx.enter_context(tc.tile_pool(name="scps", bufs=2, space="PSUM"))
        av1_ps = ctx.enter_context(tc.tile_pool(name="avps", bufs=2, space="PSUM"))
        epool = ctx.enter_context(tc.tile_pool(name="E", bufs=16))
        ofpool = ctx.enter_context(tc.tile_pool(name="ofull", bufs=2))
        rbpool = ctx.enter_context(tc.tile_pool(name="rb", bufs=2))
        w16pool = ctx.enter_context(tc.tile_pool(name="w16", bufs=2))
        crpool = ctx.enter_context(tc.tile_pool(name="crow", bufs=1))
        scrpool = ctx.enter_context(tc.tile_pool(name="scr", bufs=2))
        cspool = ctx.enter_context(tc.tile_pool(name="cs", bufs=2))
        lspool = ctx.enter_context(tc.tile_pool(name="ls", bufs=2))
        smpool = ctx.enter_context(tc.tile_pool(name="sm", bufs=4))
        vdpool = ctx.enter_context(tc.tile_pool(name="vd", bufs=2))
        for h in range(HPC):
            e_t = []
            csA2 = cspool.tile([128, 2 * NKB], dt.float32, tag="csA2")
            av1 = [av1_ps.tile([65, 512], dt.float32, tag="av", name=f"av{h}_{qb}")
                   for qb in range(2)]
            for kb in range(NKB):
                et = epool.tile([128, S], dt.float16, tag="E", name=f"E{h}_{kb}")
                e_t.append(et)
                kcols = slice(kb * 128, (kb + 1) * 128)
                for half in range(2):
                    sc = score_ps.tile([128, 1024], dt.float32, tag="sc")
                    for qq in range(2):
                        qs = slice((half * 2 + qq) * 512, (half * 2 + qq) * 512 + 512)
                        nc.tensor.matmul(sc[:, qq * 512:(qq + 1) * 512],
                                         kstack[h][:, kcols], qdup[h][:, qs],
                                         start=True, stop=True)
                    nc.scalar.activation(
                        et[:, half * 1024:(half + 1) * 1024], sc[:], F.Exp,
                        bias=0.0, scale=0.125,
                        accum_out=csA2[:, 2 * kb + half:2 * kb + half + 1])
                # AV1 for qb 0,1 accumulates during the scores pass
                for qb in range(2):
                    nc.tensor.matmul(av1[qb][:], v_aug[h][:, kb, :],
                                     e_t[kb][:, qb * 512:(qb + 1) * 512],
                                     start=(kb == 0), stop=(kb == NKB - 1))

            # drain av1 qb 0,1 -> o_full, then run qb 2,3
            o_full = ofpool.tile([65, S], dt.float32, tag="of", name=f"of{h}")
            for qb in range(2):
                nc.scalar.copy(o_full[:, qb * 512:(qb + 1) * 512], av1[qb][:])
            av2r = [av1_ps.tile([65, 512], dt.float32, tag="av", name=f"av2_{h}_{qb}")
                    for qb in range(2)]
            for kb in range(NKB):
                for qb in range(2):
                    nc.tensor.matmul(av2r[qb][:], v_aug[h][:, kb, :],
                                     e_t[kb][:, (2 + qb) * 512:(3 + qb) * 512],
                                     start=(kb == 0), stop=(kb == NKB - 1))
            for qb in range(2):
                nc.scalar.copy(o_full[:, (2 + qb) * 512:(3 + qb) * 512],
                               av2r[qb][:])

            # rb = approx 1/l broadcast to 128 partitions; rbar; w16 = r - rbar
            rb = rbpool.tile([128, S], dt.float32, tag="rb", name=f"rb{h}")
            l_sb = lspool.tile([1, S], dt.float32, tag="lsf")
            nc.sync.dma_start(l_sb[:], o_full[64:65, :])
            l128 = cspool.tile([128, 16], dt.float32, tag="l128")
            nc.sync.dma_start(l128[:], l_sb[0:1, :])
            r128 = cspool.tile([128, 16], dt.float32, tag="r128")
            nc.vector.reciprocal(r128[:], l128[:])
            r_sb = lspool.tile([1, S], dt.float32, tag="lsf")
            nc.sync.dma_start(r_sb[0:1, :], r128[:])
            for qb in range(NQ):
                qs = slice(qb * 512, (qb + 1) * 512)
                ps_l = misc_ps.tile([128, 512], dt.float32, tag="m")
                nc.tensor.matmul(ps_l[:], ones_1x128[:], r_sb[0:1, qs],
                                 start=True, stop=True)
                nc.scalar.copy(rb[:, qs], ps_l[:])
            rmax = smpool.tile([128, 1], dt.float32, tag="s1")
            nc.vector.tensor_reduce(rmax[:], rb[:], axis=AX.X, op=A.max)
            rbar = smpool.tile([128, 1], dt.float32, tag="s1")
            nc.vector.tensor_scalar(out=rbar[:], in0=rmax[:], scalar1=0.97,
                                    scalar2=None, op0=A.mult)
            w16 = w16pool.tile([128, S], dt.float16, tag="w16", name=f"w16{h}")
            nc.vector.tensor_scalar(out=w16[:], in0=rb[:], scalar1=rbar[:, 0:1],
                                    scalar2=None, op0=A.subtract)

            # compensated colsum: cs = rbar*csA + sum_q E*(r-rbar)
            csB = cspool.tile([128, NKB], dt.float32, tag="csB")
            for kb in range(NKB):
                scr = scrpool.tile([128, S], dt.float16, tag="sc16")
                nc.vector.tensor_tensor(out=scr[:], in0=e_t[kb][:], in1=w16[:],
                                        op=A.mult)
                nc.vector.reduce_sum(csB[:, kb:kb + 1], scr[:], axis=AX.X)
            cs = cspool.tile([128, 32], dt.float32, tag="cs")
            nc.vector.tensor_tensor(out=cs[:, 0:NKB],
                                    in0=csA2[:, 0:2 * NKB:2],
                                    in1=csA2[:, 1:2 * NKB:2], op=A.add)
            nc.vector.tensor_scalar(out=cs[:, 0:NKB], in0=cs[:, 0:NKB],
                                    scalar1=rbar[:, 0:1], scalar2=None, op0=A.mult)
            nc.vector.tensor_tensor(out=cs[:, 0:NKB], in0=cs[:, 0:NKB],
                                    in1=csB[:], op=A.add)

            # C_row: transpose cs -> [16,128] then broadcast rows -> [128, 2048]
            c_row = crpool.tile([128, S], dt.float32, tag="cr")
            ps_t = misc_ps.tile([128, 512], dt.float32, tag="m")
            nc.tensor.transpose(ps_t[0:16, 0:128], cs[:, 0:NKB], ident_t[:])
            csT = cspool.tile([16, 128], dt.float32, tag="csT")
            nc.scalar.copy(csT[:], ps_t[0:16, 0:128])
            cs_flat = lspool.tile([1, S], dt.float32, tag="lsf")
            nc.sync.dma_start(cs_flat[0:1, :], csT[:])
            for ch in range(4):
                ps_c = misc_ps.tile([128, 512], dt.float32, tag="m")
                nc.tensor.matmul(ps_c[:], ones_1x128[:],
                                 cs_flat[0:1, ch * 512:(ch + 1) * 512],
                                 start=True, stop=True)
                nc.scalar.copy(c_row[:, ch * 512:(ch + 1) * 512], ps_c[:])

            # vectorized bisection: 128 thresholds/partition per phase
            lo = smpool.tile([128, 1], dt.float32, tag="s1")
            nc.vector.memset(lo[:], 0.0)
            step = smpool.tile([128, 1], dt.float32, tag="s1")
            nc.vector.tensor_reduce(step[:], c_row[:], axis=AX.X, op=A.max)
            nc.vector.tensor_scalar(out=step[:], in0=step[:], scalar1=1.0 / 129.0,
                                    scalar2=None, op0=A.mult)
            for ph in range(NPH):
                T = smpool.tile([128, 1], dt.float32, tag="s1")
                nc.vector.tensor_scalar(out=T[:], in0=iota_t[:],
                                        scalar1=step[:, 0:1], scalar2=lo[:, 0:1],
                                        op0=A.mult, op1=A.add)
                scb = scrpool.tile([128, S], dt.float16, tag="sc16")
                cnt = smpool.tile([128, 1], dt.float32, tag="s1")
                nc.vector.tensor_scalar(out=scb[:], in0=c_row[:],
                                        scalar1=T[:, 0:1], scalar2=None,
                                        op0=A.is_gt, op1=A.add, accum_out=cnt[:])
                ge = smpool.tile([128, 1], dt.float32, tag="s1")
                nc.vector.tensor_scalar(out=ge[:], in0=cnt[:], scalar1=NDROP_THR,
                                        scalar2=None, op0=A.is_gt)
                m_t = smpool.tile([128, 1], dt.float32, tag="s1")
                nc.gpsimd.partition_all_reduce(m_t[:], ge[:], channels=128,
                                               reduce_op=bass_isa.ReduceOp.add)
                lo2 = smpool.tile([128, 1], dt.float32, tag="s1")
                nc.vector.tensor_scalar(out=lo2[:], in0=m_t[:],
                                        scalar1=step[:, 0:1], scalar2=lo[:, 0:1],
                                        op0=A.mult, op1=A.add)
                lo = lo2
                step2 = smpool.tile([128, 1], dt.float32, tag="s1")
                nc.vector.tensor_scalar(out=step2[:], in0=step[:],
                                        scalar1=1.0 / 129.0, scalar2=None,
                                        op0=A.mult)
                step = step2
            thr = smpool.tile([128, 1], dt.float32, tag="s1")
            nc.vector.tensor_scalar(out=thr[:], in0=step[:], scalar1=64.5,
                                    scalar2=lo[:, 0:1], op0=A.mult, op1=A.add)

            m_keep = cspool.tile([128, NKB], dt.float32, tag="mk")
            nc.vector.tensor_scalar(out=m_keep[:], in0=cs[:, 0:NKB],
                                    scalar1=thr[:, 0:1], scalar2=None, op0=A.is_gt)
            m_drop = cspool.tile([128, NKB], dt.float32, tag="md")
            nc.vector.tensor_scalar(out=m_drop[:], in0=m_keep[:], scalar1=-1.0,
                                    scalar2=1.0, op0=A.mult, op1=A.add)

            # complement AV2 over dropped columns; O = (O_full - C) * r
            vd = vdpool.tile([128, NKB, 64], dt.float16, tag="vd", name=f"vd{h}")
            for kb in range(NKB):
                nc.vector.tensor_scalar(out=vd[:, kb, :], in0=v_aug[h][:, kb, 0:64],
                                        scalar1=m_drop[:, kb:kb + 1], scalar2=None,
                                        op0=A.mult)
            tile_idx, row0 = h // 2, (h % 2) * 64
            for qb in range(NQ):
                qs = slice(qb * 512, (qb + 1) * 512)
                cps = misc_ps.tile([128, 512], dt.float32, tag="m")
                for kb in range(NKB):
                    nc.tensor.matmul(cps[0:64, :], vd[:, kb, :],
                                     e_t[kb][:, qs],
                                     start=(kb == 0), stop=(kb == NKB - 1))
                t1 = scrpool.tile([64, 512], dt.float32, tag="t1")
                nc.vector.tensor_tensor(out=t1[:], in0=o_full[0:64, qs],
                                        in1=cps[0:64, :], op=A.subtract)
                if row0 == 0:
                    nc.vector.tensor_tensor(out=ocat[tile_idx][0:64, qs],
                                            in0=t1[:], in1=rb[0:64, qs], op=A.mult)
                else:
                    t2 = scrpool.tile([64, 512], dt.float16, tag="t2")
                    nc.vector.tensor_tensor(out=t2[:], in0=t1[:],
                                            in1=rb[0:64, qs], op=A.mult)
                    nc.sync.dma_start(ocat[tile_idx][64:128, qs], t2[:])

        # ---------------- Phase O: out-projection partial ----------------
        with tc.tile_pool(name="oW", bufs=1) as wopool, \
             tc.tile_pool(name="oS", bufs=2) as ospool:
            wo_t = wopool.tile([128, 2 * DM], dt.float16)
            for ct in range(2):
                nc.sync.dma_start(wo_t[:, ct * DM:(ct + 1) * DM],
                                  wo[ct * 128:(ct + 1) * 128, :])
            for ot in range(DM // 128):
                for qb in range(NQ):
                    pso = misc_ps.tile([128, 512], dt.float32, tag="m")
                    for ct in range(2):
                        nc.tensor.matmul(
                            pso[:],
                            wo_t[:, ct * DM + ot * 128: ct * DM + ot * 128 + 128],
                            ocat[ct][:, qb * 512:(qb + 1) * 512],
                            start=(ct == 0), stop=(ct == 1))
                    osb = ospool.tile([128, 512], dt.float32, tag="osb")
                    nc.scalar.copy(osb[:], pso[:])
                    nc.sync.dma_start(out_part[ot * 128:(ot + 1) * 128,
                                               qb * 512:(qb + 1) * 512], osb[:])
    nc.compile()
    return nc


def _get_nc():
    if "nc" not in _CACHE:
        nc = bacc_mod.Bacc('TRN2', target_bir_lowering=False)
        _emit(nc)
        _CACHE["nc"] = nc
    return _CACHE["nc"]


def _split16(x):
    hi = x.astype(np.float16)
    lo = (x - hi.astype(np.float32)).astype(np.float16)
    return hi, lo


def kernel(q, k, v, Wq, bq, Wk, bk, Wv, bv, Wo, bo):
    q, k, v = (np.asarray(a, np.float32) for a in (q, k, v))
    Wq, bq, Wk, bk, Wv, bv, Wo, bo = (np.asarray(a, np.float32) for a in
                                      (Wq, bq, Wk, bk, Wv, bv, Wo, bo))
    nc = _get_nc()

    xt = {}
    for b in range(B):
        xq16 = np.ascontiguousarray(q[b].T).astype(np.float16)
        kh, kl = _split16(np.ascontiguousarray(k[b].T))
        xv16 = np.ascontiguousarray(v[b].T).astype(np.float16)
        xt[b] = (xq16, kh, kl, xv16)

    iota1 = np.arange(1, 129, dtype=np.float32).reshape(128, 1)
    ident = np.eye(128, dtype=np.float32)

    in_maps = []
    for core in range(N_CORES):
        b = core // 4
        h0 = (core % 4) * HPC
        cols = slice(h0 * DK, (h0 + HPC) * DK)
        xq16, kh, kl, xv16 = xt[b]
        wkh_, wkl_ = _split16(np.ascontiguousarray(Wk[cols].T))
        in_maps.append({
            "xq": xq16, "xkh": kh, "xkl": kl, "xv": xv16,
            "wq": np.ascontiguousarray(Wq[cols].T).astype(np.float16),
            "wkh": wkh_, "wkl": wkl_,
            "wv": np.ascontiguousarray(Wv[cols].T).astype(np.float16),
            "wo": np.ascontiguousarray(Wo[:, cols].T).astype(np.float16),
            "bqi": np.ascontiguousarray(bq[cols].reshape(2, 128).T),
            "bki": np.ascontiguousarray(bk[cols].reshape(2, 128).T),
            "bvi": np.ascontiguousarray(bv[cols].reshape(1, CPC)),
            "iota1": iota1, "ident": ident,
        })

    from concourse.bass_utils import run_bass_kernel_spmd
    _CACHE["last_in_maps"] = in_maps
    res = run_bass_kernel_spmd(nc, in_maps, core_ids=list(range(N_CORES)))
    _CACHE["last_res"] = res

    out = np.zeros((B, S, DM), np.float32)
    for core in range(N_CORES):
        b = core // 4
        out[b] += res.results[core]["out_part"].T
    out += bo.reshape(1, 1, DM)
    return out


# revision 15
# speedup vs baseline: 1.4576x; 1.0683x over previous
"""Sparse (top-k pruned) multi-head attention on 8 Trainium2 NeuronCores.

Sharding: batch(2) x head-groups(4 heads) -> 8 cores.

Per core (4 heads h=0..3, pairs ct=h//2):
  P: q-proj 1-term fp16; k-proj 3-term fp16 hi/lo (precision carrier);
     v-proj 1-term fp16 -> v_aug [128,16,65] (ones col).
     Scores operands: kstack_h = [khi;klo] dims stacked, qdup_h = [q;q].
  A (per head): scores S^T = kstack^T qdup (one 128-contraction fp16 mm
     per (kb,qs)); ACT exp (scale=1/8) -> E fp16 + f32 accum csA (exact
     unnormalized colsum); AV1 = E @ [v;1] -> O_full + row sums l;
     r = approx_recip(broadcast l); colsum = rbar*csA + sum(E*(r-rbar))
     (compensated, near-f32); C_row = PE-transpose+broadcast of colsums;
     128-thresholds-per-partition bisection x4 phases -> exact top-1843
     threshold; complement AV2 = E @ (v*drop); O = (O_full - C)*r.
  O: out-projection partial (tensor-parallel on d_model), f32 partials.
Host sums per-core partials and adds bo.
"""
import sys
for p in ('/opt/trn_rl_repo', '/opt/pypackages'):
    if p not in sys.path:
        sys.path.insert(0, p)
import numpy as np
from contextlib import ExitStack

import concourse.bass as bass
import concourse.bacc as bacc_mod
import concourse.bass_isa as bass_isa
import concourse.tile as tile
import concourse.mybir as mybir
from concourse import library_config

dt = mybir.dt
F = mybir.ActivationFunctionType
A = mybir.AluOpType
AX = mybir.AxisListType

B, S, DM, H, DK = 2, 2048, 1024, 16, 64
HPC = 4              # heads per core
CPC = HPC * DK       # 256 proj columns per core
KEEP = int(S * 0.9)  # 1843
N_CORES = 8
KT = DM // 128       # 8 contraction tiles for projections
NQ = S // 512        # 4 query chunks
NKB = S // 128       # 16 key tiles
NPH = 4              # bisection phases (129-way each)
NDROP_THR = float(KEEP) - 0.5   # count(c > t) target boundary: 1842.5

_CACHE = {}


def _emit(nc):
    ei = lambda n, s, d: nc.dram_tensor(n, s, d, kind="ExternalInput")
    xq = ei("xq", [DM, S], dt.float16)
    xkh = ei("xkh", [DM, S], dt.float16)
    xkl = ei("xkl", [DM, S], dt.float16)
    xv = ei("xv", [DM, S], dt.float16)
    wq = ei("wq", [DM, CPC], dt.float16)
    wkh = ei("wkh", [DM, CPC], dt.float16)
    wkl = ei("wkl", [DM, CPC], dt.float16)
    wv = ei("wv", [DM, CPC], dt.float16)
    wo = ei("wo", [CPC, DM], dt.float16)
    bqi = ei("bqi", [128, 2], dt.float32)
    bki = ei("bki", [128, 2], dt.float32)
    bvi = ei("bvi", [1, CPC], dt.float32)
    iota_in = ei("iota1", [128, 1], dt.float32)   # values 1..128
    ident_in = ei("ident", [128, 128], dt.float32)
    out_part = nc.dram_tensor("out_part", [DM, S], dt.float32, kind="ExternalOutput")

    with tile.TileContext(nc) as tc, ExitStack() as ctx:
        nc.gpsimd.load_library(library_config.attn)

        # ---------- long-lived pools ----------
        cpool = ctx.enter_context(tc.tile_pool(name="const", bufs=1))
        qkpool = ctx.enter_context(tc.tile_pool(name="qk", bufs=1))
        vpool = ctx.enter_context(tc.tile_pool(name="vaug", bufs=1))
        ocpool = ctx.enter_context(tc.tile_pool(name="ocat", bufs=1))
        # psum pools (total <= 8 banks): scores 2x2 + av1 2x1 + misc 2x1
        misc_ps = ctx.enter_context(tc.tile_pool(name="mps", bufs=2, space="PSUM"))

        ones_1x128 = cpool.tile([1, 128], dt.float32, tag="c1")
        nc.vector.memset(ones_1x128[:], 1.0)
        iota_t = cpool.tile([128, 1], dt.float32, tag="c2")
        nc.sync.dma_start(iota_t[:], iota_in[:, :])
        ident_t = cpool.tile([128, 128], dt.float32, tag="c3")
        nc.sync.dma_start(ident_t[:], ident_in[:, :])
        bq_sb = cpool.tile([128, 2], dt.float32, tag="c4")
        nc.sync.dma_start(bq_sb[:], bqi[:, :])
        bk_sb = cpool.tile([128, 2], dt.float32, tag="c5")
        nc.sync.dma_start(bk_sb[:], bki[:, :])

        qdup = [qkpool.tile([128, S], dt.float16, tag=f"qd{h}", name=f"qd{h}")
                for h in range(HPC)]
        kstack = [qkpool.tile([128, S], dt.float16, tag=f"ks{h}", name=f"ks{h}")
                  for h in range(HPC)]
        v_aug = [vpool.tile([128, NKB, 65], dt.float16, tag=f"va{h}", name=f"va{h}")
                 for h in range(HPC)]
        ocat = [ocpool.tile([128, S], dt.float16, tag=f"oc{i}", name=f"oc{i}")
                for i in range(2)]

        # ---------------- Phase P: projections ----------------
        with tc.tile_pool(name="pW", bufs=1) as wpool, \
             tc.tile_pool(name="pX", bufs=8) as xpool, \
             tc.tile_pool(name="pT", bufs=2) as tpool, \
             tc.tile_pool(name="pPs", bufs=4, space="PSUM") as pj_ps:

            wq_t = wpool.tile([128, KT * CPC], dt.float16)
            wkh_t = wpool.tile([128, KT * CPC], dt.float16)
            wkl_t = wpool.tile([128, KT * CPC], dt.float16)
            wv_t = wpool.tile([128, KT * CPC], dt.float16)
            for kt in range(KT):
                sl = slice(kt * CPC, (kt + 1) * CPC)
                rows = slice(kt * 128, (kt + 1) * 128)
                nc.sync.dma_start(wq_t[:, sl], wq[rows, :])
                nc.sync.dma_start(wkh_t[:, sl], wkh[rows, :])
                nc.sync.dma_start(wkl_t[:, sl], wkl[rows, :])
                nc.sync.dma_start(wv_t[:, sl], wv[rows, :])
            bv_sb = wpool.tile([1, CPC], dt.float32)
            nc.sync.dma_start(bv_sb[:], bvi[:, :])
            ps_bv = misc_ps.tile([128, 512], dt.float32, tag="m")
            nc.tensor.matmul(ps_bv[:, 0:CPC], ones_1x128[:], bv_sb[:],
                             start=True, stop=True)
            bv_bc = wpool.tile([128, CPC], dt.float32)
            nc.vector.tensor_copy(bv_bc[:], ps_bv[:, 0:CPC])

            # ---- q projection (1-term) + k projection (3-term hi/lo) ----
            xq_t = []
            xkh_t = []
            xkl_t = []
            for kt in range(KT):
                rows = slice(kt * 128, (kt + 1) * 128)
                t1 = xpool.tile([128, S], dt.float16, tag="xq", name=f"xq{kt}")
                nc.sync.dma_start(t1[:], xq[rows, :])
                xq_t.append(t1)
                t2 = xpool.tile([128, S], dt.float16, tag="xkh", name=f"xkh{kt}")
                nc.sync.dma_start(t2[:], xkh[rows, :])
                xkh_t.append(t2)
                t3 = xpool.tile([128, S], dt.float16, tag="xkl", name=f"xkl{kt}")
                nc.sync.dma_start(t3[:], xkl[rows, :])
                xkl_t.append(t3)
            for nt in range(NQ):
                qs = slice(nt * 512, (nt + 1) * 512)
                for ct in range(2):
                    hA, hB = 2 * ct, 2 * ct + 1
                    # q: single term
                    psq = pj_ps.tile([128, 512], dt.float32, tag="pj")
                    for kt in range(KT):
                        wsl = slice(kt * CPC + ct * 128, kt * CPC + ct * 128 + 128)
                        nc.tensor.matmul(psq[:], wq_t[:, wsl], xq_t[kt][:, qs],
                                         start=(kt == 0), stop=(kt == KT - 1))
                    # qdup: rows 0:64 then DMA-duplicate to rows 64:128
                    nc.vector.tensor_scalar(
                        out=qdup[hA][0:64, qs], in0=psq[0:64, :],
                        scalar1=bq_sb[0:64, ct:ct + 1], scalar2=None, op0=A.add)
                    nc.vector.tensor_scalar(
                        out=qdup[hB][0:64, qs], in0=psq[64:128, :],
                        scalar1=bq_sb[64:128, ct:ct + 1], scalar2=None, op0=A.add)
                    nc.sync.dma_start(qdup[hA][64:128, qs], qdup[hA][0:64, qs])
                    nc.sync.dma_start(qdup[hB][64:128, qs], qdup[hB][0:64, qs])

                    # k: 3 terms -> f32 psum
                    psk = pj_ps.tile([128, 512], dt.float32, tag="pj")
                    i_mm = 0
                    for kt in range(KT):
                        wsl = slice(kt * CPC + ct * 128, kt * CPC + ct * 128 + 128)
                        for (wt, xt) in ((wkh_t, xkh_t[kt]), (wkh_t, xkl_t[kt]),
                                         (wkl_t, xkh_t[kt])):
                            nc.tensor.matmul(psk[:], wt[:, wsl], xt[:, qs],
                                             start=(i_mm == 0),
                                             stop=(i_mm == 3 * KT - 1))
                            i_mm += 1
                    # kA: hi aligned rows 0:64; lo via tmp + shift-DMA
                    nc.vector.tensor_scalar(
                        out=kstack[hA][0:64, qs], in0=psk[0:64, :],
                        scalar1=bk_sb[0:64, ct:ct + 1], scalar2=None, op0=A.add)
                    tfA = tpool.tile([128, 512], dt.float32, tag="tf")
                    nc.vector.tensor_scalar(
                        out=tfA[0:64, :], in0=psk[0:64, :],
                        scalar1=bk_sb[0:64, ct:ct + 1], scalar2=None, op0=A.add)
                    tA16 = tpool.tile([128, 512], dt.float16, tag="t16")
                    nc.vector.tensor_tensor(out=tA16[0:64, :], in0=tfA[0:64, :],
                                            in1=kstack[hA][0:64, qs], op=A.subtract)
                    nc.sync.dma_start(kstack[hA][64:128, qs], tA16[0:64, :])
                    # kB: hi into tmp rows 64:128 (aligned), shift-DMA to rows 0:64;
                    #     lo aligned rows 64:128
                    tB16 = tpool.tile([128, 512], dt.float16, tag="t16")
                    nc.vector.tensor_scalar(
                        out=tB16[64:128, :], in0=psk[64:128, :],
                        scalar1=bk_sb[64:128, ct:ct + 1], scalar2=None, op0=A.add)
                    nc.sync.dma_start(kstack[hB][0:64, qs], tB16[64:128, :])
                    tfB = tpool.tile([128, 512], dt.float32, tag="tf")
                    nc.vector.tensor_scalar(
                        out=tfB[64:128, :], in0=psk[64:128, :],
                        scalar1=bk_sb[64:128, ct:ct + 1], scalar2=None, op0=A.add)
                    nc.vector.tensor_tensor(out=kstack[hB][64:128, qs],
                                            in0=tfB[64:128, :], in1=tB16[64:128, :],
                                            op=A.subtract)

            # ---- v projection (1-term fp16) ----
            xv_t = []
            for kt in range(KT):
                rows = slice(kt * 128, (kt + 1) * 128)
                t4 = xpool.tile([128, S], dt.float16, tag="xv", name=f"xv{kt}")
                nc.sync.dma_start(t4[:], xv[rows, :])
                xv_t.append(t4)
            for tb in range(NKB):
                tsl = slice(tb * 128, (tb + 1) * 128)
                psv = pj_ps.tile([128, 512], dt.float32, tag="pj")
                for kt in range(KT):
                    nc.tensor.matmul(psv[:, 0:CPC], xv_t[kt][:, tsl],
                                     wv_t[:, kt * CPC:(kt + 1) * CPC],
                                     start=(kt == 0), stop=(kt == KT - 1))
                for h in range(HPC):
                    nc.vector.tensor_tensor(
                        out=v_aug[h][:, tb, 0:64], in0=psv[:, h * 64:(h + 1) * 64],
                        in1=bv_bc[:, h * 64:(h + 1) * 64], op=A.add)
            for h in range(HPC):
                nc.vector.memset(v_aug[h][:, :, 64:65], 1.0)

        # ---------------- Phase A: per-head attention ----------------
        score_ps = ctx.enter_context(tc.tile_pool(name="scps", bufs=2, space="PSUM"))
        av1_ps = ctx.enter_context(tc.tile_pool(name="avps", bufs=2, space="PSUM"))
        epool = ctx.enter_context(tc.tile_pool(name="E", bufs=24))
        ofpool = ctx.enter_context(tc.tile_pool(name="ofull", bufs=1))
        rbpool = ctx.enter_context(tc.tile_pool(name="rb", bufs=1))
        w16pool = ctx.enter_context(tc.tile_pool(name="w16", bufs=1))
        crpool = ctx.enter_context(tc.tile_pool(name="crow", bufs=1))
        scrpool = ctx.enter_context(tc.tile_pool(name="scr", bufs=1))
        cspool = ctx.enter_context(tc.tile_pool(name="cs", bufs=1))
        lspool = ctx.enter_context(tc.tile_pool(name="ls", bufs=1))
        smpool = ctx.enter_context(tc.tile_pool(name="sm", bufs=4))
        vdpool = ctx.enter_context(tc.tile_pool(name="vd", bufs=1))
        for h in range(HPC):
            e_t = []
            csA2 = cspool.tile([128, 2 * NKB], dt.float32, tag="csA2")
            av1 = [av1_ps.tile([65, 512], dt.float32, tag="av", name=f"av{h}_{qb}")
                   for qb in range(2)]
            for kb in range(NKB):
                et = epool.tile([128, S], dt.float16, tag="E", name=f"E{h}_{kb}")
                e_t.append(et)
                kcols = slice(kb * 128, (kb + 1) * 128)
                for half in range(2):
                    sc = score_ps.tile([128, 1024], dt.float32, tag="sc")
                    for qq in range(2):
                        qs = slice((half * 2 + qq) * 512, (half * 2 + qq) * 512 + 512)
                        nc.tensor.matmul(sc[:, qq * 512:(qq + 1) * 512],
                                         kstack[h][:, kcols], qdup[h][:, qs],
                                         start=True, stop=True)
                    nc.scalar.activation(
                        et[:, half * 1024:(half + 1) * 1024], sc[:], F.Exp,
                        bias=0.0, scale=0.125,
                        accum_out=csA2[:, 2 * kb + half:2 * kb + half + 1])
                # AV1 for qb 0,1 accumulates during the scores pass
                for qb in range(2):
                    nc.tensor.matmul(av1[qb][:], v_aug[h][:, kb, :],
                                     e_t[kb][:, qb * 512:(qb + 1) * 512],
                                     start=(kb == 0), stop=(kb == NKB - 1))

            # drain av1 qb 0,1 -> o_full, then run qb 2,3
            o_full = ofpool.tile([65, S], dt.float32, tag="of", name=f"of{h}")
            for qb in range(2):
                nc.scalar.copy(o_full[:, qb * 512:(qb + 1) * 512], av1[qb][:])
            av2r = [av1_ps.tile([65, 512], dt.float32, tag="av", name=f"av2_{h}_{qb}")
                    for qb in range(2)]
            for kb in range(NKB):
                for qb in range(2):
                    nc.tensor.matmul(av2r[qb][:], v_aug[h][:, kb, :],
                                     e_t[kb][:, (2 + qb) * 512:(3 + qb) * 512],
                                     start=(kb == 0), stop=(kb == NKB - 1))
            for qb in range(2):
                nc.scalar.copy(o_full[:, (2 + qb) * 512:(3 + qb) * 512],
                               av2r[qb][:])

            # rb = approx 1/l broadcast to 128 partitions; rbar; w16 = r - rbar
            rb = rbpool.tile([128, S], dt.float32, tag="rb", name=f"rb{h}")
            l_sb = lspool.tile([1, S], dt.float32, tag="lsf")
            nc.sync.dma_start(l_sb[:], o_full[64:65, :])
            l128 = cspool.tile([128, 16], dt.float32, tag="l128")
            nc.sync.dma_start(l128[:], l_sb[0:1, :])
            r128 = cspool.tile([128, 16], dt.float32, tag="r128")
            nc.vector.reciprocal(r128[:], l128[:])
            r_sb = lspool.tile([1, S], dt.float32, tag="lsf")
            nc.sync.dma_start(r_sb[0:1, :], r128[:])
            for qb in range(NQ):
                qs = slice(qb * 512, (qb + 1) * 512)
                ps_l = misc_ps.tile([128, 512], dt.float32, tag="m")
                nc.tensor.matmul(ps_l[:], ones_1x128[:], r_sb[0:1, qs],
                                 start=True, stop=True)
                nc.scalar.copy(rb[:, qs], ps_l[:])
            rmax = smpool.tile([128, 1], dt.float32, tag="s1")
            nc.vector.tensor_reduce(rmax[:], rb[:], axis=AX.X, op=A.max)
            rbar = smpool.tile([128, 1], dt.float32, tag="s1")
            nc.vector.tensor_scalar(out=rbar[:], in0=rmax[:], scalar1=0.97,
                                    scalar2=None, op0=A.mult)
            w16 = w16pool.tile([128, S], dt.float16, tag="w16", name=f"w16{h}")
            nc.vector.tensor_scalar(out=w16[:], in0=rb[:], scalar1=rbar[:, 0:1],
                                    scalar2=None, op0=A.subtract)

            # compensated colsum: cs = rbar*csA + sum_q E*(r-rbar)
            csB = cspool.tile([128, NKB], dt.float32, tag="csB")
            for kb in range(NKB):
                scr = scrpool.tile([128, S], dt.float16, tag="sc16")
                nc.vector.scalar_tensor_tensor(
                    out=scr[:], in0=e_t[kb][:], scalar=0.0, in1=w16[:],
                    op0=A.add, op1=A.mult, accum_out=csB[:, kb:kb + 1])
            cs = cspool.tile([128, 32], dt.float32, tag="cs")
            nc.vector.tensor_tensor(out=cs[:, 0:NKB],
                                    in0=csA2[:, 0:2 * NKB:2],
                                    in1=csA2[:, 1:2 * NKB:2], op=A.add)
            nc.vector.tensor_scalar(out=cs[:, 0:NKB], in0=cs[:, 0:NKB],
                                    scalar1=rbar[:, 0:1], scalar2=None, op0=A.mult)
            nc.vector.tensor_tensor(out=cs[:, 0:NKB], in0=cs[:, 0:NKB],
                                    in1=csB[:], op=A.add)

            # C_row: transpose cs -> [16,128] then broadcast rows -> [128, 2048]
            c_row = crpool.tile([128, S], dt.float32, tag="cr")
            ps_t = misc_ps.tile([128, 512], dt.float32, tag="m")
            nc.tensor.transpose(ps_t[0:16, 0:128], cs[:, 0:NKB], ident_t[:])
            csT = cspool.tile([16, 128], dt.float32, tag="csT")
            nc.scalar.copy(csT[:], ps_t[0:16, 0:128])
            cs_flat = lspool.tile([1, S], dt.float32, tag="lsf")
            nc.sync.dma_start(cs_flat[0:1, :], csT[:])
            for ch in range(4):
                ps_c = misc_ps.tile([128, 512], dt.float32, tag="m")
                nc.tensor.matmul(ps_c[:], ones_1x128[:],
                                 cs_flat[0:1, ch * 512:(ch + 1) * 512],
                                 start=True, stop=True)
                nc.scalar.copy(c_row[:, ch * 512:(ch + 1) * 512], ps_c[:])

            # vectorized bisection: 128 thresholds/partition per phase
            lo = smpool.tile([128, 1], dt.float32, tag="s1")
            nc.vector.memset(lo[:], 0.0)
            step = smpool.tile([128, 1], dt.float32, tag="s1")
            nc.vector.tensor_reduce(step[:], c_row[:], axis=AX.X, op=A.max)
            nc.vector.tensor_scalar(out=step[:], in0=step[:], scalar1=1.0 / 129.0,
                                    scalar2=None, op0=A.mult)
            for ph in range(NPH):
                T = smpool.tile([128, 1], dt.float32, tag="s1")
                nc.vector.tensor_scalar(out=T[:], in0=iota_t[:],
                                        scalar1=step[:, 0:1], scalar2=lo[:, 0:1],
                                        op0=A.mult, op1=A.add)
                scb = scrpool.tile([128, S], dt.float16, tag="sc16")
                cnt = smpool.tile([128, 1], dt.float32, tag="s1")
                nc.vector.tensor_scalar(out=scb[:], in0=c_row[:],
                                        scalar1=T[:, 0:1], scalar2=None,
                                        op0=A.is_gt, op1=A.add, accum_out=cnt[:])
                ge = smpool.tile([128, 1], dt.float32, tag="s1")
                nc.vector.tensor_scalar(out=ge[:], in0=cnt[:], scalar1=NDROP_THR,
                                        scalar2=None, op0=A.is_gt)
                m_t = smpool.tile([128, 1], dt.float32, tag="s1")
                nc.gpsimd.partition_all_reduce(m_t[:], ge[:], channels=128,
                                               reduce_op=bass_isa.ReduceOp.add)
                lo2 = smpool.tile([128, 1], dt.float32, tag="s1")
                nc.vector.tensor_scalar(out=lo2[:], in0=m_t[:],
                                        scalar1=step[:, 0:1], scalar2=lo[:, 0:1],
                                        op0=A.mult, op1=A.add)
                lo = lo2
                step2 = smpool.tile([128, 1], dt.float32, tag="s1")
                nc.vector.tensor_scalar(out=step2[:], in0=step[:],
                                        scalar1=1.0 / 129.0, scalar2=None,
                                        op0=A.mult)
                step = step2
            thr = smpool.tile([128, 1], dt.float32, tag="s1")
            nc.vector.tensor_scalar(out=thr[:], in0=step[:], scalar1=64.5,
                                    scalar2=lo[:, 0:1], op0=A.mult, op1=A.add)

            m_keep = cspool.tile([128, NKB], dt.float32, tag="mk")
            nc.vector.tensor_scalar(out=m_keep[:], in0=cs[:, 0:NKB],
                                    scalar1=thr[:, 0:1], scalar2=None, op0=A.is_gt)
            m_drop = cspool.tile([128, NKB], dt.float32, tag="md")
            nc.vector.tensor_scalar(out=m_drop[:], in0=m_keep[:], scalar1=-1.0,
                                    scalar2=1.0, op0=A.mult, op1=A.add)

            # complement AV2 over dropped columns; O = (O_full - C) * r
            vd = vdpool.tile([128, NKB, 64], dt.float16, tag="vd", name=f"vd{h}")
            for kb in range(NKB):
                nc.gpsimd.tensor_scalar(out=vd[:, kb, :], in0=v_aug[h][:, kb, 0:64],
                                        scalar1=m_drop[:, kb:kb + 1], scalar2=None,
                                        op0=A.mult)
            tile_idx, row0 = h // 2, (h % 2) * 64
            for qb in range(NQ):
                qs = slice(qb * 512, (qb + 1) * 512)
                cps = misc_ps.tile([128, 512], dt.float32, tag="m")
                for kb in range(NKB):
                    nc.tensor.matmul(cps[0:64, :], vd[:, kb, :],
                                     e_t[kb][:, qs],
                                     start=(kb == 0), stop=(kb == NKB - 1))
                t1 = scrpool.tile([64, 512], dt.float32, tag="t1")
                nc.vector.tensor_tensor(out=t1[:], in0=o_full[0:64, qs],
                                        in1=cps[0:64, :], op=A.subtract)
                if row0 == 0:
                    nc.vector.tensor_tensor(out=ocat[tile_idx][0:64, qs],
                                            in0=t1[:], in1=rb[0:64, qs], op=A.mult)
                else:
                    t2 = scrpool.tile([64, 512], dt.float16, tag="t2")
                    nc.vector.tensor_tensor(out=t2[:], in0=t1[:],
                                            in1=rb[0:64, qs], op=A.mult)
                    nc.sync.dma_start(ocat[tile_idx][64:128, qs], t2[:])

        # ---------------- Phase O: out-projection partial ----------------
        with tc.tile_pool(name="oW", bufs=1) as wopool, \
             tc.tile_pool(name="oS", bufs=2) as ospool:
            wo_t = wopool.tile([128, 2 * DM], dt.float16)
            for ct in range(2):
                nc.sync.dma_start(wo_t[:, ct * DM:(ct + 1) * DM],
                                  wo[ct * 128:(ct + 1) * 128, :])
            for ot in range(DM // 128):
                for qb in range(NQ):
                    pso = misc_ps.tile([128, 512], dt.float32, tag="m")
                    for ct in range(2):
                        nc.tensor.matmul(
                            pso[:],
                            wo_t[:, ct * DM + ot * 128: ct * DM + ot * 128 + 128],
                            ocat[ct][:, qb * 512:(qb + 1) * 512],
                            start=(ct == 0), stop=(ct == 1))
                    osb = ospool.tile([128, 512], dt.float32, tag="osb")
                    nc.scalar.copy(osb[:], pso[:])
                    nc.sync.dma_start(out_part[ot * 128:(ot + 1) * 128,
                                               qb * 512:(qb + 1) * 512], osb[:])
    nc.compile()
    return nc


def _get_nc():
    if "nc" not in _CACHE:
        nc = bacc_mod.Bacc('TRN2', target_bir_lowering=False)
        _emit(nc)
        _CACHE["nc"] = nc
    return _CACHE["nc"]


def _split16(x):
    hi = x.astype(np.float16)
    lo = (x - hi.astype(np.float32)).astype(np.float16)
    return hi, lo


def kernel(q, k, v, Wq, bq, Wk, bk, Wv, bv, Wo, bo):
    q, k, v = (np.asarray(a, np.float32) for a in (q, k, v))
    Wq, bq, Wk, bk, Wv, bv, Wo, bo = (np.asarray(a, np.float32) for a in
                                      (Wq, bq, Wk, bk, Wv, bv, Wo, bo))
    nc = _get_nc()

    xt = {}
    for b in range(B):
        xq16 = np.ascontiguousarray(q[b].T).astype(np.float16)
        kh, kl = _split16(np.ascontiguousarray(k[b].T))
        xv16 = np.ascontiguousarray(v[b].T).astype(np.float16)
        xt[b] = (xq16, kh, kl, xv16)

    iota1 = np.arange(1, 129, dtype=np.float32).reshape(128, 1)
    ident = np.eye(128, dtype=np.float32)

    in_maps = []
    for core in range(N_CORES):
        b = core // 4
        h0 = (core % 4) * HPC
        cols = slice(h0 * DK, (h0 + HPC) * DK)
        xq16, kh, kl, xv16 = xt[b]
        wkh_, wkl_ = _split16(np.ascontiguousarray(Wk[cols].T))
        in_maps.append({
            "xq": xq16, "xkh": kh, "xkl": kl, "xv": xv16,
            "wq": np.ascontiguousarray(Wq[cols].T).astype(np.float16),
            "wkh": wkh_, "wkl": wkl_,
            "wv": np.ascontiguousarray(Wv[cols].T).astype(np.float16),
            "wo": np.ascontiguousarray(Wo[:, cols].T).astype(np.float16),
            "bqi": np.ascontiguousarray(bq[cols].reshape(2, 128).T),
            "bki": np.ascontiguousarray(bk[cols].reshape(2, 128).T),
            "bvi": np.ascontiguousarray(bv[cols].reshape(1, CPC)),
            "iota1": iota1, "ident": ident,
        })

    from concourse.bass_utils import run_bass_kernel_spmd
    _CACHE["last_in_maps"] = in_maps
    res = run_bass_kernel_spmd(nc, in_maps, core_ids=list(range(N_CORES)))
    _CACHE["last_res"] = res

    out = np.zeros((B, S, DM), np.float32)
    for core in range(N_CORES):
        b = core // 4
        out[b] += res.results[core]["out_part"].T
    out += bo.reshape(1, 1, DM)
    return out


# revision 18
# speedup vs baseline: 1.5465x; 1.0610x over previous
"""Sparse (top-k pruned) multi-head attention on 8 Trainium2 NeuronCores.

Sharding: batch(2) x head-groups(4 heads) -> 8 cores.

Per core (4 heads h=0..3, pairs ct=h//2):
  P: q-proj 1-term fp16; k-proj 3-term fp16 hi/lo (precision carrier);
     v-proj 1-term fp16 -> v_aug [128,16,65] (ones col).
     Scores operands: kstack_h = [khi;klo] dims stacked, qdup_h = [q;q].
  A (per head): scores S^T = kstack^T qdup (one 128-contraction fp16 mm
     per (kb,qs)); ACT exp (scale=1/8) -> E fp16 + f32 accum csA (exact
     unnormalized colsum); AV1 = E @ [v;1] -> O_full + row sums l;
     r = approx_recip(broadcast l); colsum = rbar*csA + sum(E*(r-rbar))
     (compensated, near-f32); C_row = PE-transpose+broadcast of colsums;
     128-thresholds-per-partition bisection x4 phases -> exact top-1843
     threshold; complement AV2 = E @ (v*drop); O = (O_full - C)*r.
  O: out-projection partial (tensor-parallel on d_model), f32 partials.
Host sums per-core partials and adds bo.
"""
import sys
for p in ('/opt/trn_rl_repo', '/opt/pypackages'):
    if p not in sys.path:
        sys.path.insert(0, p)
import numpy as np
from contextlib import ExitStack

import concourse.bass as bass
import concourse.bacc as bacc_mod
import concourse.bass_isa as bass_isa
import concourse.tile as tile
import concourse.mybir as mybir
from concourse import library_config

dt = mybir.dt
F = mybir.ActivationFunctionType
A = mybir.AluOpType
AX = mybir.AxisListType

B, S, DM, H, DK = 2, 2048, 1024, 16, 64
HPC = 4              # heads per core
CPC = HPC * DK       # 256 proj columns per core
KEEP = int(S * 0.9)  # 1843
N_CORES = 8
KT = DM // 128       # 8 contraction tiles for projections
NQ = S // 512        # 4 query chunks
NKB = S // 128       # 16 key tiles
NPH = 4              # bisection phases (129-way each)
NDROP_THR = float(KEEP) - 0.5   # count(c > t) target boundary: 1842.5

_CACHE = {}


def _emit(nc):
    ei = lambda n, s, d: nc.dram_tensor(n, s, d, kind="ExternalInput")
    xq = ei("xq", [DM, S], dt.float16)
    xkh = ei("xkh", [DM, S], dt.float16)
    xkl = ei("xkl", [DM, S], dt.float16)
    xv = ei("xv", [DM, S], dt.float16)
    wq = ei("wq", [DM, CPC], dt.float16)
    wkh = ei("wkh", [DM, CPC], dt.float16)
    wkl = ei("wkl", [DM, CPC], dt.float16)
    wv = ei("wv", [DM, CPC], dt.float16)
    wo = ei("wo", [CPC, DM], dt.float16)
    bqi = ei("bqi", [128, 2], dt.float32)
    bki = ei("bki", [128, 2], dt.float32)
    bvi = ei("bvi", [1, CPC], dt.float32)
    iota_in = ei("iota1", [128, 1], dt.float32)   # values 1..128
    ident_in = ei("ident", [128, 128], dt.float32)
    out_part = nc.dram_tensor("out_part", [DM, S], dt.float32, kind="ExternalOutput")

    with tile.TileContext(nc) as tc, ExitStack() as ctx:
        nc.gpsimd.load_library(library_config.attn)

        # ---------- long-lived pools ----------
        cpool = ctx.enter_context(tc.tile_pool(name="const", bufs=1))
        qkpool = ctx.enter_context(tc.tile_pool(name="qk", bufs=1))
        vpool = ctx.enter_context(tc.tile_pool(name="vaug", bufs=1))
        ocpool = ctx.enter_context(tc.tile_pool(name="ocat", bufs=1))
        # psum pools (total <= 8 banks): scores 2x2 + av1 2x1 + misc 2x1
        misc_ps = ctx.enter_context(tc.tile_pool(name="mps", bufs=2, space="PSUM"))

        ones_1x128 = cpool.tile([1, 128], dt.float32, tag="c1")
        nc.vector.memset(ones_1x128[:], 1.0)
        iota_t = cpool.tile([128, 1], dt.float32, tag="c2")
        nc.sync.dma_start(iota_t[:], iota_in[:, :])
        ident_t = cpool.tile([128, 128], dt.float32, tag="c3")
        nc.sync.dma_start(ident_t[:], ident_in[:, :])
        bq_sb = cpool.tile([128, 2], dt.float32, tag="c4")
        nc.sync.dma_start(bq_sb[:], bqi[:, :])
        bk_sb = cpool.tile([128, 2], dt.float32, tag="c5")
        nc.sync.dma_start(bk_sb[:], bki[:, :])

        qdup = [qkpool.tile([128, S], dt.float16, tag=f"qd{h}", name=f"qd{h}")
                for h in range(HPC)]
        kstack = [qkpool.tile([128, S], dt.float16, tag=f"ks{h}", name=f"ks{h}")
                  for h in range(HPC)]
        v_aug = [vpool.tile([128, NKB, 65], dt.float16, tag=f"va{h}", name=f"va{h}")
                 for h in range(HPC)]
        ocat = [ocpool.tile([128, S], dt.float16, tag=f"oc{i}", name=f"oc{i}")
                for i in range(2)]

        # ---------------- Phase P: projections ----------------
        with tc.tile_pool(name="pW", bufs=1) as wpool, \
             tc.tile_pool(name="pX", bufs=8) as xpool, \
             tc.tile_pool(name="pT", bufs=2) as tpool, \
             tc.tile_pool(name="pPs", bufs=4, space="PSUM") as pj_ps:

            wq_t = wpool.tile([128, KT * CPC], dt.float16)
            wkh_t = wpool.tile([128, KT * CPC], dt.float16)
            wkl_t = wpool.tile([128, KT * CPC], dt.float16)
            wv_t = wpool.tile([128, KT * CPC], dt.float16)
            for kt in range(KT):
                sl = slice(kt * CPC, (kt + 1) * CPC)
                rows = slice(kt * 128, (kt + 1) * 128)
                nc.sync.dma_start(wq_t[:, sl], wq[rows, :])
                nc.sync.dma_start(wkh_t[:, sl], wkh[rows, :])
                nc.sync.dma_start(wkl_t[:, sl], wkl[rows, :])
                nc.sync.dma_start(wv_t[:, sl], wv[rows, :])
            bv_sb = wpool.tile([1, CPC], dt.float32)
            nc.sync.dma_start(bv_sb[:], bvi[:, :])
            ps_bv = misc_ps.tile([128, 512], dt.float32, tag="m")
            nc.tensor.matmul(ps_bv[:, 0:CPC], ones_1x128[:], bv_sb[:],
                             start=True, stop=True)
            bv_bc = wpool.tile([128, CPC], dt.float32)
            nc.vector.tensor_copy(bv_bc[:], ps_bv[:, 0:CPC])

            # ---- q projection (1-term) + k projection (3-term hi/lo) ----
            xq_t = []
            xkh_t = []
            xkl_t = []
            for kt in range(KT):
                rows = slice(kt * 128, (kt + 1) * 128)
                t1 = xpool.tile([128, S], dt.float16, tag="xq", name=f"xq{kt}")
                nc.sync.dma_start(t1[:], xq[rows, :])
                xq_t.append(t1)
                t2 = xpool.tile([128, S], dt.float16, tag="xkh", name=f"xkh{kt}")
                nc.sync.dma_start(t2[:], xkh[rows, :])
                xkh_t.append(t2)
                t3 = xpool.tile([128, S], dt.float16, tag="xkl", name=f"xkl{kt}")
                nc.sync.dma_start(t3[:], xkl[rows, :])
                xkl_t.append(t3)
            for nt in range(NQ):
                qs = slice(nt * 512, (nt + 1) * 512)
                for ct in range(2):
                    hA, hB = 2 * ct, 2 * ct + 1
                    # q: single term
                    psq = pj_ps.tile([128, 512], dt.float32, tag="pj")
                    for kt in range(KT):
                        wsl = slice(kt * CPC + ct * 128, kt * CPC + ct * 128 + 128)
                        nc.tensor.matmul(psq[:], wq_t[:, wsl], xq_t[kt][:, qs],
                                         start=(kt == 0), stop=(kt == KT - 1))
                    # qdup: rows 0:64 then DMA-duplicate to rows 64:128
                    nc.vector.tensor_scalar(
                        out=qdup[hA][0:64, qs], in0=psq[0:64, :],
                        scalar1=bq_sb[0:64, ct:ct + 1], scalar2=None, op0=A.add)
                    nc.vector.tensor_scalar(
                        out=qdup[hB][0:64, qs], in0=psq[64:128, :],
                        scalar1=bq_sb[64:128, ct:ct + 1], scalar2=None, op0=A.add)
                    nc.sync.dma_start(qdup[hA][64:128, qs], qdup[hA][0:64, qs])
                    nc.sync.dma_start(qdup[hB][64:128, qs], qdup[hB][0:64, qs])

                    # k: 3 terms -> f32 psum
                    psk = pj_ps.tile([128, 512], dt.float32, tag="pj")
                    i_mm = 0
                    for kt in range(KT):
                        wsl = slice(kt * CPC + ct * 128, kt * CPC + ct * 128 + 128)
                        for (wt, xt) in ((wkh_t, xkh_t[kt]), (wkh_t, xkl_t[kt]),
                                         (wkl_t, xkh_t[kt])):
                            nc.tensor.matmul(psk[:], wt[:, wsl], xt[:, qs],
                                             start=(i_mm == 0),
                                             stop=(i_mm == 3 * KT - 1))
                            i_mm += 1
                    # kA: hi aligned rows 0:64; lo via tmp + shift-DMA
                    nc.vector.tensor_scalar(
                        out=kstack[hA][0:64, qs], in0=psk[0:64, :],
                        scalar1=bk_sb[0:64, ct:ct + 1], scalar2=None, op0=A.add)
                    tfA = tpool.tile([128, 512], dt.float32, tag="tf")
                    nc.vector.tensor_scalar(
                        out=tfA[0:64, :], in0=psk[0:64, :],
                        scalar1=bk_sb[0:64, ct:ct + 1], scalar2=None, op0=A.add)
                    tA16 = tpool.tile([128, 512], dt.float16, tag="t16")
                    nc.vector.tensor_tensor(out=tA16[0:64, :], in0=tfA[0:64, :],
                                            in1=kstack[hA][0:64, qs], op=A.subtract)
                    nc.sync.dma_start(kstack[hA][64:128, qs], tA16[0:64, :])
                    # kB: hi into tmp rows 64:128 (aligned), shift-DMA to rows 0:64;
                    #     lo aligned rows 64:128
                    tB16 = tpool.tile([128, 512], dt.float16, tag="t16")
                    nc.vector.tensor_scalar(
                        out=tB16[64:128, :], in0=psk[64:128, :],
                        scalar1=bk_sb[64:128, ct:ct + 1], scalar2=None, op0=A.add)
                    nc.sync.dma_start(kstack[hB][0:64, qs], tB16[64:128, :])
                    tfB = tpool.tile([128, 512], dt.float32, tag="tf")
                    nc.vector.tensor_scalar(
                        out=tfB[64:128, :], in0=psk[64:128, :],
                        scalar1=bk_sb[64:128, ct:ct + 1], scalar2=None, op0=A.add)
                    nc.vector.tensor_tensor(out=kstack[hB][64:128, qs],
                                            in0=tfB[64:128, :], in1=tB16[64:128, :],
                                            op=A.subtract)

            # ---- v projection (1-term fp16) ----
            xv_t = []
            for kt in range(KT):
                rows = slice(kt * 128, (kt + 1) * 128)
                t4 = xpool.tile([128, S], dt.float16, tag="xv", name=f"xv{kt}")
                nc.sync.dma_start(t4[:], xv[rows, :])
                xv_t.append(t4)
            for tb in range(NKB):
                tsl = slice(tb * 128, (tb + 1) * 128)
                psv = pj_ps.tile([128, 512], dt.float32, tag="pj")
                for kt in range(KT):
                    nc.tensor.matmul(psv[:, 0:CPC], xv_t[kt][:, tsl],
                                     wv_t[:, kt * CPC:(kt + 1) * CPC],
                                     start=(kt == 0), stop=(kt == KT - 1))
                for h in range(HPC):
                    nc.vector.tensor_tensor(
                        out=v_aug[h][:, tb, 0:64], in0=psv[:, h * 64:(h + 1) * 64],
                        in1=bv_bc[:, h * 64:(h + 1) * 64], op=A.add)
            for h in range(HPC):
                nc.vector.memset(v_aug[h][:, :, 64:65], 1.0)

        # ---------------- Phase A: per-head attention ----------------
        score_ps = ctx.enter_context(tc.tile_pool(name="scps", bufs=2, space="PSUM"))
        av1_ps = ctx.enter_context(tc.tile_pool(name="avps", bufs=2, space="PSUM"))
        epool = ctx.enter_context(tc.tile_pool(name="E", bufs=24))
        ofpool = ctx.enter_context(tc.tile_pool(name="ofull", bufs=1))
        rbpool = ctx.enter_context(tc.tile_pool(name="rb", bufs=1))
        w16pool = ctx.enter_context(tc.tile_pool(name="w16", bufs=1))
        crpool = ctx.enter_context(tc.tile_pool(name="crow", bufs=1))
        scrpool = ctx.enter_context(tc.tile_pool(name="scr", bufs=1))
        cspool = ctx.enter_context(tc.tile_pool(name="cs", bufs=1))
        lspool = ctx.enter_context(tc.tile_pool(name="ls", bufs=1))
        smpool = ctx.enter_context(tc.tile_pool(name="sm", bufs=4))
        vdpool = ctx.enter_context(tc.tile_pool(name="vd", bufs=1))

        hctx = {}

        def emit_scores(h):
            e_t = []
            csA2 = cspool.tile([128, 2 * NKB], dt.float32, tag="csA2",
                               name=f"csA2_{h}")
            av1 = [av1_ps.tile([65, 512], dt.float32, tag="av", name=f"av{h}_{qb}")
                   for qb in range(2)]
            for kb in range(NKB):
                et = epool.tile([128, S], dt.float16, tag="E", name=f"E{h}_{kb}")
                e_t.append(et)
                kcols = slice(kb * 128, (kb + 1) * 128)
                for half in range(2):
                    sc = score_ps.tile([128, 1024], dt.float32, tag="sc")
                    for qq in range(2):
                        qs = slice((half * 2 + qq) * 512, (half * 2 + qq) * 512 + 512)
                        nc.tensor.matmul(sc[:, qq * 512:(qq + 1) * 512],
                                         kstack[h][:, kcols], qdup[h][:, qs],
                                         start=True, stop=True)
                    nc.scalar.activation(
                        et[:, half * 1024:(half + 1) * 1024], sc[:], F.Exp,
                        bias=0.0, scale=0.125,
                        accum_out=csA2[:, 2 * kb + half:2 * kb + half + 1])
                for qb in range(2):
                    nc.tensor.matmul(av1[qb][:], v_aug[h][:, kb, :],
                                     e_t[kb][:, qb * 512:(qb + 1) * 512],
                                     start=(kb == 0), stop=(kb == NKB - 1))
            hctx[h] = dict(e_t=e_t, csA2=csA2, av1=av1)

        def emit_tail(h):
            e_t = hctx[h]["e_t"]
            csA2 = hctx[h]["csA2"]
            av1 = hctx[h]["av1"]
            # drain av1 qb 0,1 -> o_full, then run qb 2,3
            o_full = ofpool.tile([65, S], dt.float32, tag="of", name=f"of{h}")
            for qb in range(2):
                nc.scalar.copy(o_full[:, qb * 512:(qb + 1) * 512], av1[qb][:])
            av2r = [misc_ps.tile([65, 512], dt.float32, tag="m",
                                 name=f"av2_{h}_{qb}") for qb in range(2)]
            for kb in range(NKB):
                for qb in range(2):
                    nc.tensor.matmul(av2r[qb][:], v_aug[h][:, kb, :],
                                     e_t[kb][:, (2 + qb) * 512:(3 + qb) * 512],
                                     start=(kb == 0), stop=(kb == NKB - 1))
            for qb in range(2):
                nc.scalar.copy(o_full[:, (2 + qb) * 512:(3 + qb) * 512],
                               av2r[qb][:])

            # r = 1/l via [128,16] reshape; broadcast to rb
            rb = rbpool.tile([128, S], dt.float32, tag="rb", name=f"rb{h}")
            l_sb = lspool.tile([1, S], dt.float32, tag="lsf")
            nc.sync.dma_start(l_sb[:], o_full[64:65, :])
            l128 = cspool.tile([128, 16], dt.float32, tag="l128")
            nc.sync.dma_start(l128[:], l_sb[0:1, :])
            r128 = cspool.tile([128, 16], dt.float32, tag="r128")
            nc.vector.reciprocal(r128[:], l128[:])
            r_sb = lspool.tile([1, S], dt.float32, tag="lsf")
            nc.sync.dma_start(r_sb[0:1, :], r128[:])
            for qb in range(NQ):
                qs = slice(qb * 512, (qb + 1) * 512)
                ps_l = misc_ps.tile([128, 512], dt.float32, tag="m")
                nc.tensor.matmul(ps_l[:], ones_1x128[:], r_sb[0:1, qs],
                                 start=True, stop=True)
                nc.scalar.copy(rb[:, qs], ps_l[:])
            rmax = smpool.tile([128, 1], dt.float32, tag="s1")
            nc.vector.tensor_reduce(rmax[:], rb[:], axis=AX.X, op=A.max)
            rbar = smpool.tile([128, 1], dt.float32, tag="s1")
            nc.vector.tensor_scalar(out=rbar[:], in0=rmax[:], scalar1=0.97,
                                    scalar2=None, op0=A.mult)
            w16 = w16pool.tile([128, S], dt.float16, tag="w16", name=f"w16{h}")
            nc.vector.tensor_scalar(out=w16[:], in0=rb[:], scalar1=rbar[:, 0:1],
                                    scalar2=None, op0=A.subtract)

            # compensated colsum, split across DVE and GpSimd
            csB = cspool.tile([128, NKB], dt.float32, tag="csB")
            for kb in range(NKB):
                scr = scrpool.tile([128, S], dt.float16, tag="sc16")
                nc.vector.scalar_tensor_tensor(
                    out=scr[:], in0=e_t[kb][:], scalar=0.0, in1=w16[:],
                    op0=A.add, op1=A.mult, accum_out=csB[:, kb:kb + 1])
            cs = cspool.tile([128, 32], dt.float32, tag="cs")
            nc.vector.tensor_tensor(out=cs[:, 0:NKB],
                                    in0=csA2[:, 0:2 * NKB:2],
                                    in1=csA2[:, 1:2 * NKB:2], op=A.add)
            nc.vector.tensor_scalar(out=cs[:, 0:NKB], in0=cs[:, 0:NKB],
                                    scalar1=rbar[:, 0:1], scalar2=None, op0=A.mult)
            nc.vector.tensor_tensor(out=cs[:, 0:NKB], in0=cs[:, 0:NKB],
                                    in1=csB[:], op=A.add)

            # C_row: transpose cs, flatten, broadcast
            c_row = crpool.tile([128, S], dt.float32, tag="cr")
            ps_t = misc_ps.tile([128, 512], dt.float32, tag="m")
            nc.tensor.transpose(ps_t[0:16, 0:128], cs[:, 0:NKB], ident_t[:])
            csT = cspool.tile([16, 128], dt.float32, tag="csT")
            nc.scalar.copy(csT[:], ps_t[0:16, 0:128])
            cs_flat = lspool.tile([1, S], dt.float32, tag="lsf")
            nc.sync.dma_start(cs_flat[0:1, :], csT[:])
            for ch in range(4):
                ps_c = misc_ps.tile([128, 512], dt.float32, tag="m")
                nc.tensor.matmul(ps_c[:], ones_1x128[:],
                                 cs_flat[0:1, ch * 512:(ch + 1) * 512],
                                 start=True, stop=True)
                nc.scalar.copy(c_row[:, ch * 512:(ch + 1) * 512], ps_c[:])

            # vectorized bisection: 128 thresholds/partition per phase
            lo = smpool.tile([128, 1], dt.float32, tag="s1")
            nc.vector.memset(lo[:], 0.0)
            step = smpool.tile([128, 1], dt.float32, tag="s1")
            nc.vector.tensor_reduce(step[:], c_row[:], axis=AX.X, op=A.max)
            nc.vector.tensor_scalar(out=step[:], in0=step[:], scalar1=1.0 / 129.0,
                                    scalar2=None, op0=A.mult)
            for ph in range(NPH):
                T = smpool.tile([128, 1], dt.float32, tag="s1")
                nc.vector.tensor_scalar(out=T[:], in0=iota_t[:],
                                        scalar1=step[:, 0:1], scalar2=lo[:, 0:1],
                                        op0=A.mult, op1=A.add)
                scb = scrpool.tile([128, S], dt.float16, tag="sc16")
                cnt = smpool.tile([128, 1], dt.float32, tag="s1")
                nc.vector.tensor_scalar(out=scb[:], in0=c_row[:],
                                        scalar1=T[:, 0:1], scalar2=None,
                                        op0=A.is_gt, op1=A.add, accum_out=cnt[:])
                ge = smpool.tile([128, 1], dt.float32, tag="s1")
                nc.vector.tensor_scalar(out=ge[:], in0=cnt[:], scalar1=NDROP_THR,
                                        scalar2=None, op0=A.is_gt)
                m_t = smpool.tile([128, 1], dt.float32, tag="s1")
                nc.gpsimd.partition_all_reduce(m_t[:], ge[:], channels=128,
                                               reduce_op=bass_isa.ReduceOp.add)
                lo2 = smpool.tile([128, 1], dt.float32, tag="s1")
                nc.vector.tensor_scalar(out=lo2[:], in0=m_t[:],
                                        scalar1=step[:, 0:1], scalar2=lo[:, 0:1],
                                        op0=A.mult, op1=A.add)
                lo = lo2
                step2 = smpool.tile([128, 1], dt.float32, tag="s1")
                nc.vector.tensor_scalar(out=step2[:], in0=step[:],
                                        scalar1=1.0 / 129.0, scalar2=None,
                                        op0=A.mult)
                step = step2
            thr = smpool.tile([128, 1], dt.float32, tag="s1")
            nc.vector.tensor_scalar(out=thr[:], in0=step[:], scalar1=64.5,
                                    scalar2=lo[:, 0:1], op0=A.mult, op1=A.add)

            m_keep = cspool.tile([128, NKB], dt.float32, tag="mk")
            nc.vector.tensor_scalar(out=m_keep[:], in0=cs[:, 0:NKB],
                                    scalar1=thr[:, 0:1], scalar2=None, op0=A.is_gt)
            m_drop = cspool.tile([128, NKB], dt.float32, tag="md")
            nc.vector.tensor_scalar(out=m_drop[:], in0=m_keep[:], scalar1=-1.0,
                                    scalar2=1.0, op0=A.mult, op1=A.add)

            # complement AV2 over dropped columns; O = (O_full - C) * r
            vd = vdpool.tile([128, NKB, 64], dt.float16, tag="vd", name=f"vd{h}")
            for kb in range(NKB):
                nc.vector.tensor_scalar(out=vd[:, kb, :], in0=v_aug[h][:, kb, 0:64],
                                        scalar1=m_drop[:, kb:kb + 1], scalar2=None,
                                        op0=A.mult)
            tile_idx, row0 = h // 2, (h % 2) * 64
            for qb in range(NQ):
                qs = slice(qb * 512, (qb + 1) * 512)
                cps = misc_ps.tile([128, 512], dt.float32, tag="m")
                for kb in range(NKB):
                    nc.tensor.matmul(cps[0:64, :], vd[:, kb, :],
                                     e_t[kb][:, qs],
                                     start=(kb == 0), stop=(kb == NKB - 1))
                t1 = scrpool.tile([64, 512], dt.float32, tag="t1")
                nc.vector.tensor_tensor(out=t1[:], in0=o_full[0:64, qs],
                                        in1=cps[0:64, :], op=A.subtract)
                if row0 == 0:
                    nc.vector.tensor_tensor(out=ocat[tile_idx][0:64, qs],
                                            in0=t1[:], in1=rb[0:64, qs], op=A.mult)
                else:
                    t2 = scrpool.tile([64, 512], dt.float16, tag="t2")
                    nc.vector.tensor_tensor(out=t2[:], in0=t1[:],
                                            in1=rb[0:64, qs], op=A.mult)
                    nc.sync.dma_start(ocat[tile_idx][64:128, qs], t2[:])

        # software pipeline: scores(h+1) emitted before tail(h)
        emit_scores(0)
        for h in range(1, HPC):
            emit_scores(h)
            emit_tail(h - 1)
        emit_tail(HPC - 1)

        # ---------------- Phase O: out-projection partial ----------------
        with tc.tile_pool(name="oW", bufs=1) as wopool, \
             tc.tile_pool(name="oS", bufs=2) as ospool:
            wo_t = wopool.tile([128, 2 * DM], dt.float16)
            for ct in range(2):
                nc.sync.dma_start(wo_t[:, ct * DM:(ct + 1) * DM],
                                  wo[ct * 128:(ct + 1) * 128, :])
            for ot in range(DM // 128):
                for qb in range(NQ):
                    pso = misc_ps.tile([128, 512], dt.float32, tag="m")
                    for ct in range(2):
                        nc.tensor.matmul(
                            pso[:],
                            wo_t[:, ct * DM + ot * 128: ct * DM + ot * 128 + 128],
                            ocat[ct][:, qb * 512:(qb + 1) * 512],
                            start=(ct == 0), stop=(ct == 1))
                    osb = ospool.tile([128, 512], dt.float32, tag="osb")
                    nc.scalar.copy(osb[:], pso[:])
                    nc.sync.dma_start(out_part[ot * 128:(ot + 1) * 128,
                                               qb * 512:(qb + 1) * 512], osb[:])
    nc.compile()
    return nc


def _get_nc():
    if "nc" not in _CACHE:
        nc = bacc_mod.Bacc('TRN2', target_bir_lowering=False)
        _emit(nc)
        _CACHE["nc"] = nc
    return _CACHE["nc"]


def _split16(x):
    hi = x.astype(np.float16)
    lo = (x - hi.astype(np.float32)).astype(np.float16)
    return hi, lo


def kernel(q, k, v, Wq, bq, Wk, bk, Wv, bv, Wo, bo):
    q, k, v = (np.asarray(a, np.float32) for a in (q, k, v))
    Wq, bq, Wk, bk, Wv, bv, Wo, bo = (np.asarray(a, np.float32) for a in
                                      (Wq, bq, Wk, bk, Wv, bv, Wo, bo))
    nc = _get_nc()

    xt = {}
    for b in range(B):
        xq16 = np.ascontiguousarray(q[b].T).astype(np.float16)
        kh, kl = _split16(np.ascontiguousarray(k[b].T))
        xv16 = np.ascontiguousarray(v[b].T).astype(np.float16)
        xt[b] = (xq16, kh, kl, xv16)

    iota1 = np.arange(1, 129, dtype=np.float32).reshape(128, 1)
    ident = np.eye(128, dtype=np.float32)

    in_maps = []
    for core in range(N_CORES):
        b = core // 4
        h0 = (core % 4) * HPC
        cols = slice(h0 * DK, (h0 + HPC) * DK)
        xq16, kh, kl, xv16 = xt[b]
        wkh_, wkl_ = _split16(np.ascontiguousarray(Wk[cols].T))
        in_maps.append({
            "xq": xq16, "xkh": kh, "xkl": kl, "xv": xv16,
            "wq": np.ascontiguousarray(Wq[cols].T).astype(np.float16),
            "wkh": wkh_, "wkl": wkl_,
            "wv": np.ascontiguousarray(Wv[cols].T).astype(np.float16),
            "wo": np.ascontiguousarray(Wo[:, cols].T).astype(np.float16),
            "bqi": np.ascontiguousarray(bq[cols].reshape(2, 128).T),
            "bki": np.ascontiguousarray(bk[cols].reshape(2, 128).T),
            "bvi": np.ascontiguousarray(bv[cols].reshape(1, CPC)),
            "iota1": iota1, "ident": ident,
        })

    from concourse.bass_utils import run_bass_kernel_spmd
    _CACHE["last_in_maps"] = in_maps
    res = run_bass_kernel_spmd(nc, in_maps, core_ids=list(range(N_CORES)))
    _CACHE["last_res"] = res

    out = np.zeros((B, S, DM), np.float32)
    for core in range(N_CORES):
        b = core // 4
        out[b] += res.results[core]["out_part"].T
    out += bo.reshape(1, 1, DM)
    return out
